# revision 41
# baseline (speedup 1.0000x reference)
"""Trainium2 Bass kernel: multi-head attention (B=2, N=2048, DIM=768, H=12, Dh=64),
sharded (batch x head-group) across 8 NeuronCores. Self-contained.

fp8 fast paths: the QK projection and the PV matmul run as fp8e4m3
DoubleRow matmuls (two contraction planes per instruction, 2 moving
cols/cycle): QK contracts kc pairs of x/w (weights pre-scaled by SQ/SK out
of e4m3's subnormal range, descaled in the on-device bias-add); PV
contracts kt pairs with P emitted in fp8 straight from the ScalarE exp and
V cast to fp8 (ones-columns stay exact). V projection, scores, and the
output projection stay bf16 for accuracy (sim: rel_err 0.015 vs 2e-2 gate).

Per-core shard (core = b*4 + g, g in 0..3, heads 3g..3g+2):
  - computes Q^T,K^T (features on partitions) and V (tokens on partitions) from x[b]^T
  - scores S^T[k,q] per head via row-tiled K=64 matmuls (2 concurrent per slot via
    tile_position partition halves; head 2 pairs its own even/odd k-tiles through a
    partition-swapped copy of Q2/K2)
  - exp on ScalarE over [128,1024] PSUM tiles shared by the head pair
  - O~^T and softmax sums in one matmul: V is augmented with 64 ones-columns so
    rows 64:128 of the accumulator hold the sums broadcast across partitions
  - normalize with a stock-op Newton reciprocal (bit-trick seed; sign fixed on host),
    project with this group's proj_w rows, partial out [2048, 768] f32
Host: shards inputs (bf16, scale folded into Wq, layouts pre-arranged), gathers:
  out[b] = -(sum_g partial_gb) + (qkv_b[v-part] @ proj_w + proj_b).
Scheduling: PE warmup spam against HAM cold-clock, QK/V phase interleaved with the
DMA arrival order and with attention(qc=0), projection delayed one chunk, per-chunk
normalize emitted after the next projection so the VectorE drains evictions first.
"""

import sys

for _p in ("/opt/trn_rl_repo",):
    if _p not in sys.path:
        sys.path.append(_p)

import numpy as np
import ml_dtypes

import concourse.bass as bass
import concourse.mybir as mybir
import concourse.tile as tile
from concourse.bass_utils import run_bass_kernel_spmd

BF16 = mybir.dt.bfloat16
F32 = mybir.dt.float32
F8 = mybir.dt.float8e4
bf16 = ml_dtypes.bfloat16
e4m3 = ml_dtypes.float8_e4m3fn

B, N, DIM = 2, 2048, 768
H, Dh = 12, 64
G = 3  # heads per core
NCORES = 8
QC = 512  # query chunk (free dim of score matmuls)
NQC = N // QC
KT = 128  # key tile (partition dim of S^T)
NKT = N // KT

# fp8 weight pre-scales (keep e4m3 operands out of the subnormal range);
# the inverse is applied in the on-device bias-add.
SQ = 2.0**7  # wq (softmax scale folded in, rms ~0.0025)
SK = 2.0**4  # wk (rms ~0.02)

# exp split: which k-tiles go to the VectorE (custom poly) vs ScalarE (table exp).
# DVE k-tiles are singles; ACT k-tiles are grouped in pairs of 2 (one [128,1024] inst).
# NOTE: custom DVE ops fail to encode in this container's walrus ("ISA wrong
# length"), so all exp goes through ScalarE for now.
DVE_KTS = ()
ACT_PAIRS = tuple((2 * i, 2 * i + 1) for i in range(8))

# EXP4 constants: exp(x) ~ C0F^4 * ((1+A x)(1 + B x + CC x^2))^4 on |x| <= 2.75
EXP_A = 0.14770726095997042
EXP_B = 0.10315315610745052
EXP_CC = 0.017226206106509708
EXP_C0F = 0.9990441257079289
ACT_BIAS = -4.0 * float(np.log(EXP_C0F))  # ScalarE computes exp(x + bias) to match

# DVE schraudolph-to-fp8 "exp": i8 = round(K*s + C) bitcast as e4m3 gives
# ~exp(s + ACT_BIAS) with 3.1% rms / 8% max multiplicative error (unbiased,
# so it mixes with exact ACT exp tiles inside one softmax row). Valid for
# s + ACT_BIAS in (-4.5, +5.5); the actual score range here is +-2.4.
SCH_K = 8.0 / float(np.log(2.0))
SCH_C = 56.0 - 0.46 + SCH_K * ACT_BIAS


# --------------------------------------------------------------------------
# workaround: this container's walrus accepts only ONE sync-wait per
# instruction ("Too many sync wait commands"). Split multi-wait sync_infos
# onto same-engine NoOps inserted right before the instruction.
def _patch_to_json():
    import orjson

    if getattr(bass.Bass, "_ant_json_patched", False):
        return
    orig = bass.Bass.to_json_bytes

    def to_json_bytes(self, *a, **kw):
        m = orjson.loads(orig(self, *a, **kw))

        def walk(o):
            if isinstance(o, dict):
                insts = o.get("instructions")
                if isinstance(insts, list) and insts and isinstance(insts[0], dict):
                    new = []
                    for inst in insts:
                        si = inst.get("sync_info")
                        waits = (si or {}).get("on_wait") or []
                        if len(waits) > 1:
                            for i, w in enumerate(waits[:-1]):
                                new.append(
                                    {
                                        "debug": inst.get("debug", 0),
                                        "engine": inst["engine"],
                                        "ins": [],
                                        "name": f"{inst['name']}-sw{i}",
                                        "opcode": "NoOp",
                                        "outs": [],
                                        "sync_info": {
                                            "on_update": [],
                                            "on_wait": [w],
                                        },
                                    }
                                )
                            si["on_wait"] = waits[-1:]
                        new.append(inst)
                    o["instructions"] = new
                for v in o.values():
                    walk(v)
            elif isinstance(o, list):
                for v in o:
                    walk(v)

        walk(m)
        return orjson.dumps(m)

    bass.Bass.to_json_bytes = to_json_bytes
    bass.Bass._ant_json_patched = True


# workaround: this container's walrus allows only 1 sync-wait on SP CTRL ops;
# Tile's kernel-tail drain piles every outstanding proc wait onto one Drain.
def _patch_tile_drain():
    from concourse.tile import TileContext, ScopedClock

    if getattr(TileContext, "_ant_drain_patched", False):
        return

    def _drain_and_barrier(self, tick_clock, wait_clock):
        nc = self.nc
        collector = nc.sync.nop(nofuse=True)
        wait_clock.add_sem_waits(
            collector.ins, ScopedClock({None: tick_clock.global_clock})
        )
        si = collector.ins.sync_info
        waits = list(si.on_wait) if si is not None else []
        if len(waits) > 1:
            si.on_wait = waits[:1]
            for w in waits[1:]:
                extra = nc.sync.nop(nofuse=True)
                extra.ins.sync_info = mybir.SyncInfo(on_wait=[w], on_update=[])
        nc.sync.drain()
        nc.all_engine_barrier()
        assert self.sems is not None
        popped = nc._tile_sem_poison_stack.pop()
        assert popped is self._sem_poison
        nc.clear_and_free_semaphores(list(self.sems.allocated().values()))
        nc.all_engine_barrier()

    TileContext._drain_and_barrier = _drain_and_barrier
    TileContext._ant_drain_patched = True


# --------------------------------------------------------------------------
# custom DVE ops: cubic ~ exp(x/4)/C0F (1 pass) and x -> x^4 (1 pass)
_EXP_OPS = {}


def _register_exp_ops():
    if _EXP_OPS:
        return _EXP_OPS
    from concourse import dve_ops
    from concourse.dve_ops import DveOp, OPS, _SUB_OPCODE_FOR_NAME
    from concourse.dve_spec import Spec, Src0, C0, C1, C2, One, sq, lower
    from concourse.dve_uop import DveOpSpec

    def make(name, spec):
        if name in _SUB_OPCODE_FOR_NAME:
            for op in OPS:
                if op.name == name:
                    return op
        row = max(_SUB_OPCODE_FOR_NAME.values()) + 1
        op = DveOp(name, spec, subdim=False, uops_sha={})
        OPS.append(op)
        _SUB_OPCODE_FOR_NAME[name] = row
        dve_ops.CUSTOM_DVE_SPECS[name] = spec
        for ver in ("v3", "v4"):
            uops = lower(spec, ver=ver)
            op.uops_sha[ver] = DveOpSpec(
                name=name, opcode=row, uops=uops, rd1_en=False
            ).sha(ver)
        return op

    cubic = make(
        "EXPC_ANT",
        Spec(
            body=(Src0 * C0 + One) * ((sq(Src0) * C2 + Src0 * C1) + One),
            reference=lambda in0, in1, s0, s1, imm2: (in0 * s0 + 1.0)
            * ((in0 * in0) * imm2 + in0 * s1 + 1.0),
        ),
    )
    pow4 = make(
        "POW4_ANT",
        Spec(
            body=sq(sq(Src0)),
            reference=lambda in0, in1, s0, s1, imm2: (in0 * in0) * (in0 * in0),
        ),
    )
    _EXP_OPS["cubic"] = cubic
    _EXP_OPS["pow4"] = pow4
    return _EXP_OPS


# --------------------------------------------------------------------------
def build_kernel():
    _patch_to_json()
    _patch_tile_drain()
    Exp = mybir.ActivationFunctionType.Exp
    Alu = mybir.AluOpType

    nc = bass.Bass(trn_type="TRN2")
    xT = nc.dram_tensor("xT", [DIM, N], BF16, kind="ExternalInput")
    xT8 = nc.dram_tensor("xT8", [DIM, N], F8, kind="ExternalInput")
    wqk = nc.dram_tensor("wqk", [DIM, 384], F8, kind="ExternalInput")
    bqk = nc.dram_tensor("bqk", [384], F32, kind="ExternalInput")
    dsc = nc.dram_tensor("dsc", [128, 3], F32, kind="ExternalInput")
    wv = nc.dram_tensor("wv", [DIM, 192], BF16, kind="ExternalInput")
    wp = nc.dram_tensor("wp", [192, DIM], BF16, kind="ExternalInput")
    out = nc.dram_tensor("out", [N, DIM], F32, kind="ExternalOutput")

    KC = DIM // 128  # 6 contraction chunks

    with tile.TileContext(nc) as tc:
        with (
            tc.tile_pool(name="persist", bufs=1) as pp,
            tc.tile_pool(name="pt_act", bufs=4) as pta,
            tc.tile_pool(name="scratch", bufs=4) as sp,
            tc.tile_pool(name="osb", bufs=3) as op_,
            tc.tile_pool(name="ysb", bufs=3) as yp,
            tc.tile_pool(name="ps", bufs=3, space="PSUM") as ps,
            tc.tile_pool(name="ps_acc", bufs=2, space="PSUM") as ps_acc,
        ):
            # ---- persistent SBUF ----
            xT_sb = pp.tile([128, KC, N], BF16, tag="xT")
            xT8_sb = pp.tile([128, KC, N], F8, tag="xT8")
            wqk_sb = pp.tile([128, KC, 384], F8, tag="wqk")
            wv_sb = pp.tile([128, KC, 192], BF16, tag="wv")
            wp_sb = pp.tile([128, 2, DIM], BF16, tag="wp")
            bqk_sb = pp.tile([128, 3], F32, tag="bqk")
            dsc_sb = pp.tile([128, 3], F32, tag="dsc")
            actbias_sb = pp.tile([128, 1], F32, tag="actbias")
            warm_sb = pp.tile([128, 8], BF16, tag="warm")
            qk_sb = pp.tile([128, 4, N], BF16, tag="qkT")  # mt: [Q0|Q1],[K0|K1],[Q2|K2],[K2d|Q2d]
            v_sb = pp.tile([128, NKT, 384], F8, tag="vaug")  # per kt: 3x [v_h(64) | ones(64)]

            # PE clock (HAM) warmup on zeroed SBUF + early exp-table load,
            # all before the heavyweight DMAs and memsets are queued.
            warm_in = pp.tile([128, 256], BF16, tag="warmmm")
            nc.gpsimd.memset(warm_in[:], 0.0)
            nc.gpsimd.memset(actbias_sb[:], ACT_BIAS)
            wps = ps.tile([128, 2 * QC], F32, tag="sa", name="warmps")[:, 0:256]
            for i in range(32):
                nc.tensor.matmul(wps[:], warm_in[:, 0:128], warm_in[:],
                                 start=(i == 0), stop=(i == 31))
            nc.scalar.activation(warm_sb[:], actbias_sb[:].to_broadcast((128, 8)), Exp)

            nc.sync.dma_start(wqk_sb[:], wqk.rearrange("(o p) m -> p o m", p=128))
            nc.sync.dma_start(bqk_sb[:], bqk.rearrange("(m p) -> p m", p=128))
            nc.sync.dma_start(dsc_sb[:], dsc[:, :])
            nc.gpsimd.dma_start(wv_sb[:], wv.rearrange("(o p) m -> p o m", p=128))
            nc.gpsimd.dma_start(wp_sb[:, 0, :], wp[0:128, :])
            nc.gpsimd.dma_start(wp_sb[0:64, 1, :], wp[128:192, :])
            # x arrives in (token-chunk, kc) granules so compute starts early:
            # fp8 copy first (feeds qk_phase), bf16 second (feeds v_tile)
            for qq in range(NQC):
                for kc in range(KC):
                    eng = nc.sync if kc % 2 == 0 else nc.gpsimd
                    eng.dma_start(
                        xT8_sb[:, kc, QC * qq : QC * qq + QC],
                        xT8[128 * kc : 128 * kc + 128, QC * qq : QC * qq + QC],
                    )
                for kc in range(KC):
                    eng = nc.gpsimd if kc % 2 == 0 else nc.sync
                    eng.dma_start(
                        xT_sb[:, kc, QC * qq : QC * qq + QC],
                        xT[128 * kc : 128 * kc + 128, QC * qq : QC * qq + QC],
                    )
            nc.gpsimd.memset(v_sb[:], 1.0)

            DR = mybir.MatmulPerfMode.DoubleRow

            def qk_phase(qc):
                # Q^T / K^T projection (fp8 DoubleRow over kc pairs) for one
                # 512-token slice, + head-2 swap copy. The fp8 weight
                # pre-scales are undone in the bias-add (dsc column).
                for mt in range(3):
                    ps_t = ps.tile([128, 2 * QC], F32, tag="sa", name="qkps")[:, 0:QC]
                    for c in range(KC // 2):
                        nc.tensor.matmul(
                            ps_t[:],
                            wqk_sb[:, 2 * c : 2 * c + 2, 128 * mt : 128 * mt + 128],
                            xT8_sb[:, 2 * c : 2 * c + 2, QC * qc : QC * qc + QC],
                            start=(c == 0),
                            stop=(c == KC // 2 - 1),
                            perf_mode=DR,
                        )
                    nc.vector.tensor_scalar(
                        qk_sb[:, mt, QC * qc : QC * qc + QC],
                        ps_t[:],
                        dsc_sb[:, mt : mt + 1],
                        bqk_sb[:, mt : mt + 1],
                        Alu.mult,
                        Alu.add,
                    )
                sl = slice(QC * qc, QC * qc + QC)
                nc.sync.dma_start(qk_sb[0:64, 3, sl], qk_sb[64:128, 2, sl])
                nc.sync.dma_start(qk_sb[64:128, 3, sl], qk_sb[0:64, 2, sl])

            def v_tile(kt):
                    ps_t = ps.tile([128, 2 * QC], F32, tag="sa", name="vps")[:, 0:192]
                    for kc in range(KC):
                        nc.tensor.matmul(
                            ps_t[:],
                            xT_sb[:, kc, KT * kt : KT * kt + KT],
                            wv_sb[:, kc, :],
                            start=(kc == 0),
                            stop=(kc == KC - 1),
                        )
                    nc.vector.tensor_copy(
                        out=v_sb[:, kt, :].rearrange("p (h c) -> p h c", c=128)[:, :, 0:64],
                        in_=ps_t[:].rearrange("p (h c) -> p h c", c=64),
                    )

            # score matmul operands: heads 0/1 pair on partition halves; head 2
            # alternates halves by kt parity via the swapped copy in slot 3.
            def s_operands(h, kt):
                if h < 2:
                    po = 64 * h
                    return (1, po), (0, po)
                return ((3, 0) if kt % 2 == 0 else (2, 64)), ((2, 0) if kt % 2 == 0 else (3, 64))

            def s_mm(dst, h, kt, qc):
                (lm, lp), (rm, rp) = s_operands(h, kt)
                nc.tensor.matmul(
                    dst,
                    qk_sb[lp : lp + 64, lm, KT * kt : KT * kt + KT],
                    qk_sb[rp : rp + 64, rm, QC * qc : QC * qc + QC],
                    start=True,
                    stop=True,
                    tile_position=(lp, 0),
                )

            def attn_begin(qc):
                return {
                    "qc": qc,
                    # oab: cols 0:QC = normalized [h0;h1] o, cols QC:2QC
                    # rows 0:64 = normalized h2 o
                    "oab": op_.tile([128, 2 * QC], BF16, tag="oab", name="oab"),
                    "ocO": sp.tile([128, QC], F32, tag="ocO", name="ocO", bufs=2),
                    "ocS": sp.tile([128, 2 * QC], F32, tag="ocS", name="ocS", bufs=2),
                    "pend": None,
                }

            def _flush(st, keep=0):
                # fp8 DoubleRow PV: kt pair (k0, k0+1) as the two planes.
                # PVs run `keep` supersteps behind their exp so the PE never
                # waits on an exp that was just issued.
                while len(st["pend"]) > keep:
                    _flush1(st, st["pend"].pop(0))

            def _flush1(st, pend):
                kind, k0, pt = pend
                if kind == "01":
                    pr = pt[:].rearrange("p (two x) -> p two x", two=2)
                    for h in (0, 1):
                        nc.tensor.matmul(
                            st["o_ps"][h][:],
                            v_sb[:, k0 : k0 + 2, 128 * h : 128 * h + 128],
                            pr[:, :, QC * h : QC * h + QC],
                            start=(k0 == 0),
                            stop=(k0 == NKT - 2),
                            perf_mode=DR,
                        )
                else:
                    nc.tensor.matmul(
                        st["o_ps"][2][:],
                        v_sb[:, k0 : k0 + 2, 256:384],
                        pt[:].rearrange("p (two x) -> p two x", two=2),
                        start=(k0 == 0),
                        stop=(k0 == NKT - 2),
                        perf_mode=DR,
                    )

            def attn_steps01(st, k0s, dve_b=(), filler=None):
                # heads 0/1, kt pairs; per superstep: 4 score matmuls (2
                # concurrent pairs), 2 exps, 2 DoubleRow PVs (one step late
                # so the PE never waits on the exp it just issued). dve_b
                # routes the kt+1 exp of those supersteps to the DVE
                # schraudolph; filler(j) emits PE filler (proj chunks).
                qc = st["qc"]
                o_ps = st.setdefault("o_ps", {})
                for h in (0, 1):
                    if h not in o_ps:
                        o_ps[h] = ps_acc.tile([128, QC], F32, tag="acc", name="acc")
                for j, k0 in enumerate(k0s):
                    s2a = ps.tile([128, 2 * QC], F32, tag="sa", name="sa")
                    s_mm(s2a[:, 0:QC], 0, k0, qc)
                    s_mm(s2a[:, QC : 2 * QC], 1, k0, qc)
                    s2b = ps.tile([128, 2 * QC], F32, tag="sa", name="sa")
                    s_mm(s2b[:, 0:QC], 0, k0 + 1, qc)
                    s_mm(s2b[:, QC : 2 * QC], 1, k0 + 1, qc)
                    pt = pta.tile([128, 4 * QC], F8, tag="pta", name="pta")
                    if j in dve_b:
                        # whole superstep on the DVE so the two engines never
                        # write the same pta tile concurrently
                        nc.vector.tensor_scalar(
                            pt[:, 0 : 2 * QC].bitcast(mybir.dt.int8), s2a[:],
                            SCH_K, SCH_C, Alu.mult, Alu.add,
                        )
                        nc.vector.tensor_scalar(
                            pt[:, 2 * QC : 4 * QC].bitcast(mybir.dt.int8), s2b[:],
                            SCH_K, SCH_C, Alu.mult, Alu.add,
                        )
                    else:
                        nc.scalar.activation(pt[:, 0 : 2 * QC], s2a[:], Exp, bias=actbias_sb[:])
                        nc.scalar.activation(pt[:, 2 * QC : 4 * QC], s2b[:], Exp, bias=actbias_sb[:])
                    st["pend"].append(("01", k0, pt))
                    _flush(st, keep=1)
                    if filler is not None:
                        filler(j)

            def attn_steps2(st, k0s, filler=None):
                # head 2: even kt on one partition half, odd on the other
                qc = st["qc"]
                o_ps = st.setdefault("o_ps", {})
                if 2 not in o_ps:
                    o_ps[2] = ps_acc.tile([128, QC], F32, tag="acc", name="acc")
                for idx, k0 in enumerate(k0s):
                    s2 = ps.tile([128, 2 * QC], F32, tag="sa", name="sa")
                    s_mm(s2[:, 0:QC], 2, k0, qc)
                    s_mm(s2[:, QC : 2 * QC], 2, k0 + 1, qc)
                    pt = pta.tile([128, 2 * QC], F8, tag="pt2", name="pt2")
                    if idx % 2 == 1:
                        # DVE "exp": schraudolph straight into the fp8 bit
                        # pattern — i8 = round(K*s + C), bitcast e4m3.
                        nc.vector.tensor_scalar(
                            pt[:].bitcast(mybir.dt.int8), s2[:],
                            SCH_K, SCH_C, Alu.mult, Alu.add,
                        )
                    else:
                        nc.scalar.activation(pt[:], s2[:], Exp, bias=actbias_sb[:])
                    st["pend"].append(("2", k0, pt))
                    _flush(st, keep=1)
                    if filler is not None:
                        filler(idx)

            def attn_evac01(st):
                # free heads 0/1 accumulators, assembling (o | sums) staging:
                # ocO = [h0 o ; h1 o], ocS cols 0:QC = [h0 sums ; h1 sums]
                _flush(st)
                o_ps, ocO, ocS = st["o_ps"], st["ocO"], st["ocS"]
                nc.vector.tensor_copy(out=ocO[0:64, :], in_=o_ps[0][0:64, :])
                nc.vector.tensor_copy(out=ocO[64:128, :], in_=o_ps[1][0:64, :])
                nc.vector.tensor_copy(out=ocS[0:64, 0:QC], in_=o_ps[0][64:128, :])
                nc.vector.tensor_copy(out=ocS[64:128, 0:QC], in_=o_ps[1][64:128, :])
                del o_ps[0], o_ps[1]

            def attn_evac2(st):
                # h2 sums into ocS cols QC:2QC (rows 64:128 stay junk); the
                # o half stays in PSUM until the normalize multiply.
                _flush(st)
                nc.vector.tensor_copy(
                    out=st["ocS"][0:64, QC : 2 * QC], in_=st["o_ps"][2][64:128, :]
                )

            MAGIC = 0x7EF311C3

            def normalize(st, last=False):
                # rec = -(approx 1/sums): int bit-trick seed + 1 Newton step on
                # the assembled [128, 2QC] sums tile (3 heads in one chain; the
                # sign is fixed up on the host, partials are negated). The
                # seed/newton chain runs on the Pool engine to keep the DVE
                # free for exp tiles.
                ocS, ocO = st["ocS"], st["ocO"]
                seedt = sp.tile([128, 2 * QC], F32, tag="seed", name="seed", bufs=2)
                nc.vector.tensor_scalar(
                    seedt[:].bitcast(mybir.dt.int32),
                    ocS[:].bitcast(mybir.dt.int32),
                    MAGIC, -1, Alu.subtract, Alu.mult,
                )
                ut = sp.tile([128, 2 * QC], F32, tag="nru", name="nru", bufs=2)
                nc.vector.tensor_tensor(ut[:], ocS[:], seedt[:], Alu.mult)
                rect = sp.tile([128, 2 * QC], F32, tag="recip", name="recip", bufs=2)
                nc.vector.scalar_tensor_tensor(
                    rect[:], ut[:], 2.0, seedt[:], Alu.subtract, Alu.mult
                )
                oab = st["oab"]
                nc.vector.tensor_tensor(oab[:, 0:QC], ocO[:], rect[:, 0:QC], Alu.mult)
                nc.vector.tensor_tensor(
                    oab[0:64, QC : 2 * QC],
                    st["o_ps"][2][0:64, :],
                    rect[0:64, QC : 2 * QC],
                    Alu.mult,
                )
                del st["o_ps"][2]

            def proj_chunk(pst, j):
                # one (qt, column-half) chunk of the output projection for
                # pst's query slice; evictions alternate ScalarE / VectorE.
                qc, oab = pst["qc"], pst["oab"]
                qt, half = divmod(j, 2)
                if half == 0:
                    pst["ys"] = yp.tile([128, DIM], F32, tag="y", name="y")
                ys = pst["ys"]
                nsl = slice(384 * half, 384 * half + 384)
                yps = ps.tile([128, 2 * QC], F32, tag="sa", name="yps")[:, 0:384]
                nc.tensor.matmul(
                    yps[:], oab[:, 128 * qt : 128 * qt + 128], wp_sb[:, 0, nsl],
                    start=True, stop=False,
                )
                nc.tensor.matmul(
                    yps[:],
                    oab[0:64, QC + 128 * qt : QC + 128 * qt + 128],
                    wp_sb[0:64, 1, nsl],
                    start=False, stop=True,
                )
                if half == 0:
                    nc.scalar.copy(ys[:, nsl], yps[:])
                else:
                    nc.vector.tensor_copy(out=ys[:, nsl], in_=yps[:])
                    nc.gpsimd.dma_start(
                        out[QC * qc + 128 * qt : QC * qc + 128 * qt + 128, :], ys[:]
                    )

            # ---- software-pipelined schedule ----
            # Phase 1 (QK/V projections) is interleaved with attention(qc=0):
            # attention consumes K^T/V k-tiles in order, and k-tile group g
            # becomes available right after qk_phase(g)+v_tile(4g..4g+3).
            qk_phase(0)
            for kt in range(0, 4):
                v_tile(kt)
            st0 = attn_begin(0)
            attn_steps01(st0, (0, 2))
            for qq in range(1, NQC):
                qk_phase(qq)
                for kt in range(4 * qq, 4 * qq + 4):
                    v_tile(kt)
                attn_steps01(st0, (4 * qq, 4 * qq + 2))
            attn_evac01(st0)
            attn_steps2(st0, range(0, NKT, 2))
            attn_evac2(st0)
            normalize(st0)
            prev = st0
            for qc in range(1, NQC):
                st = attn_begin(qc)
                attn_steps01(st, range(0, NKT, 2), dve_b=(2, 5))
                attn_evac01(st)
                # proj(qc-1) chunks interleave with the h2 steps: the PE has
                # slack there and the evictions split across ScalarE/VectorE
                attn_steps2(
                    st, range(0, NKT, 2),
                    filler=lambda i, p=prev: proj_chunk(p, i),
                )
                attn_evac2(st)
                normalize(st, last=(qc == NQC - 1))
                prev = st
            for j in range(8):
                proj_chunk(prev, j)
    return nc


_NC_CACHE = {}


def _get_nc():
    if "nc" not in _NC_CACHE:
        _NC_CACHE["nc"] = build_kernel()
    return _NC_CACHE["nc"]


def kernel(x, qkv_w, qkv_b, proj_w, proj_b):
    x = np.asarray(x, np.float32)
    qkv_w = np.asarray(qkv_w, np.float32)
    qkv_b = np.asarray(qkv_b, np.float32)
    proj_w = np.asarray(proj_w, np.float32)
    proj_b = np.asarray(proj_b, np.float32)

    wr = qkv_w.reshape(DIM, 3, H, Dh)
    br = qkv_b.reshape(3, H, Dh)
    scale = Dh ** -0.5

    # fp8 descale vector per mt slot: mt0 all-Q, mt1 all-K, mt2 [Q2|K2]
    dsc_c = np.empty((128, 3), np.float32)
    dsc_c[:, 0] = 1.0 / SQ
    dsc_c[:, 1] = 1.0 / SK
    dsc_c[0:64, 2] = 1.0 / SQ
    dsc_c[64:128, 2] = 1.0 / SK

    in_maps = []
    for core in range(NCORES):
        b, g = divmod(core, 4)
        hs = slice(G * g, G * g + G)
        # fold softmax scale into Q; pre-scale fp8 weights out of subnormals
        wq = wr[:, 0, hs, :].reshape(DIM, G * Dh) * (scale * SQ)
        wk = wr[:, 1, hs, :].reshape(DIM, G * Dh) * SK
        wvm = wr[:, 2, hs, :].reshape(DIM, G * Dh)
        bq = br[0, hs].reshape(G * Dh) * scale
        bk = br[1, hs].reshape(G * Dh)
        # column order: mt0=[Q0|Q1], mt1=[K0|K1], mt2=[Q2|K2] (64 cols per head)
        wqk_c = np.concatenate(
            [wq[:, 0:128], wk[:, 0:128], wq[:, 128:192], wk[:, 128:192]], axis=1
        )
        bqk_c = np.concatenate([bq[0:128], bk[0:128], bq[128:192], bk[128:192]])
        xTb = np.ascontiguousarray(x[b].T)
        in_maps.append(
            {
                "xT": xTb.astype(bf16),
                "xT8": xTb.astype(e4m3).view(np.uint8),
                "wqk": np.ascontiguousarray(wqk_c).astype(e4m3).view(np.uint8),
                "bqk": np.ascontiguousarray(bqk_c),
                "dsc": dsc_c,
                "wv": np.ascontiguousarray(wvm).astype(bf16),
                "wp": np.ascontiguousarray(proj_w[64 * G * g : 64 * G * (g + 1), :]).astype(bf16),
            }
        )

    nc = _get_nc()
    res = run_bass_kernel_spmd(nc, in_maps, core_ids=list(range(NCORES)))
    _NC_CACHE["last_result"] = res

    bias_row = (br[2].reshape(DIM).astype(np.float64) @ proj_w.astype(np.float64)
                + proj_b.astype(np.float64)).astype(np.float32)
    out = np.zeros((B, N, DIM), np.float32)
    for b in range(B):
        acc = np.zeros((N, DIM), np.float64)
        for g in range(4):
            acc += res.results[4 * b + g]["out"].astype(np.float64)
        out[b] = (-acc).astype(np.float32) + bias_row
    return out



# revision 42
# speedup vs baseline: 1.0770x; 1.0770x over previous
"""Trainium2 Bass kernel: multi-head attention (B=2, N=2048, DIM=768, H=12, Dh=64),
sharded (batch x head-group) across 8 NeuronCores. Self-contained.

fp8 fast paths: the QK projection and the PV matmul run as fp8e4m3
DoubleRow matmuls (two contraction planes per instruction, 2 moving
cols/cycle): QK contracts kc pairs of x/w (weights pre-scaled by SQ/SK out
of e4m3's subnormal range, descaled in the on-device bias-add); PV
contracts kt pairs with P emitted in fp8 straight from the ScalarE exp and
V cast to fp8 (ones-columns stay exact). V projection, scores, and the
output projection stay bf16 for accuracy (sim: rel_err 0.015 vs 2e-2 gate).

Per-core shard (core = b*4 + g, g in 0..3, heads 3g..3g+2):
  - computes Q^T,K^T (features on partitions) and V (tokens on partitions) from x[b]^T
  - scores S^T[k,q] per head via row-tiled K=64 matmuls (2 concurrent per slot via
    tile_position partition halves; head 2 pairs its own even/odd k-tiles through a
    partition-swapped copy of Q2/K2)
  - exp on ScalarE over [128,1024] PSUM tiles shared by the head pair
  - O~^T and softmax sums in one matmul: V is augmented with 64 ones-columns so
    rows 64:128 of the accumulator hold the sums broadcast across partitions
  - normalize with a stock-op Newton reciprocal (bit-trick seed; sign fixed on host),
    project with this group's proj_w rows, partial out [2048, 768] f32
Host: shards inputs (bf16, scale folded into Wq, layouts pre-arranged), gathers:
  out[b] = -(sum_g partial_gb) + (qkv_b[v-part] @ proj_w + proj_b).
Scheduling: PE warmup spam against HAM cold-clock, QK/V phase interleaved with the
DMA arrival order and with attention(qc=0), projection delayed one chunk, per-chunk
normalize emitted after the next projection so the VectorE drains evictions first.
"""

import sys

for _p in ("/opt/trn_rl_repo",):
    if _p not in sys.path:
        sys.path.append(_p)

import numpy as np
import ml_dtypes

import concourse.bass as bass
import concourse.mybir as mybir
import concourse.tile as tile
from concourse.bass_utils import run_bass_kernel_spmd

BF16 = mybir.dt.bfloat16
F32 = mybir.dt.float32
F8 = mybir.dt.float8e4
bf16 = ml_dtypes.bfloat16
e4m3 = ml_dtypes.float8_e4m3fn

B, N, DIM = 2, 2048, 768
H, Dh = 12, 64
G = 3  # heads per core
NCORES = 8
QC = 512  # query chunk (free dim of score matmuls)
NQC = N // QC
KT = 128  # key tile (partition dim of S^T)
NKT = N // KT

# fp8 weight pre-scales (keep e4m3 operands out of the subnormal range);
# the inverse is applied in the on-device bias-add.
SQ = 2.0**7  # wq (softmax scale folded in, rms ~0.0025)
SK = 2.0**4  # wk (rms ~0.02)

# exp split: which k-tiles go to the VectorE (custom poly) vs ScalarE (table exp).
# DVE k-tiles are singles; ACT k-tiles are grouped in pairs of 2 (one [128,1024] inst).
# NOTE: custom DVE ops fail to encode in this container's walrus ("ISA wrong
# length"), so all exp goes through ScalarE for now.
DVE_KTS = ()
ACT_PAIRS = tuple((2 * i, 2 * i + 1) for i in range(8))

# EXP4 constants: exp(x) ~ C0F^4 * ((1+A x)(1 + B x + CC x^2))^4 on |x| <= 2.75
EXP_A = 0.14770726095997042
EXP_B = 0.10315315610745052
EXP_CC = 0.017226206106509708
EXP_C0F = 0.9990441257079289
ACT_BIAS = -4.0 * float(np.log(EXP_C0F))  # ScalarE computes exp(x + bias) to match

# DVE schraudolph-to-fp8 "exp": i8 = round(K*s + C) bitcast as e4m3 gives
# ~exp(s + ACT_BIAS) with 3.1% rms / 8% max multiplicative error (unbiased,
# so it mixes with exact ACT exp tiles inside one softmax row). Valid for
# s + ACT_BIAS in (-4.5, +5.5); the actual score range here is +-2.4.
SCH_K = 8.0 / float(np.log(2.0))
SCH_C = 56.0 - 0.46 + SCH_K * ACT_BIAS


# --------------------------------------------------------------------------
# workaround: this container's walrus accepts only ONE sync-wait per
# instruction ("Too many sync wait commands"). Split multi-wait sync_infos
# onto same-engine NoOps inserted right before the instruction.
def _patch_to_json():
    import orjson

    if getattr(bass.Bass, "_ant_json_patched", False):
        return
    orig = bass.Bass.to_json_bytes

    def to_json_bytes(self, *a, **kw):
        m = orjson.loads(orig(self, *a, **kw))

        def walk(o):
            if isinstance(o, dict):
                insts = o.get("instructions")
                if isinstance(insts, list) and insts and isinstance(insts[0], dict):
                    new = []
                    for inst in insts:
                        si = inst.get("sync_info")
                        waits = (si or {}).get("on_wait") or []
                        if len(waits) > 1:
                            for i, w in enumerate(waits[:-1]):
                                new.append(
                                    {
                                        "debug": inst.get("debug", 0),
                                        "engine": inst["engine"],
                                        "ins": [],
                                        "name": f"{inst['name']}-sw{i}",
                                        "opcode": "NoOp",
                                        "outs": [],
                                        "sync_info": {
                                            "on_update": [],
                                            "on_wait": [w],
                                        },
                                    }
                                )
                            si["on_wait"] = waits[-1:]
                        new.append(inst)
                    o["instructions"] = new
                for v in o.values():
                    walk(v)
            elif isinstance(o, list):
                for v in o:
                    walk(v)

        walk(m)
        return orjson.dumps(m)

    bass.Bass.to_json_bytes = to_json_bytes
    bass.Bass._ant_json_patched = True


# workaround: this container's walrus allows only 1 sync-wait on SP CTRL ops;
# Tile's kernel-tail drain piles every outstanding proc wait onto one Drain.
def _patch_tile_drain():
    from concourse.tile import TileContext, ScopedClock

    if getattr(TileContext, "_ant_drain_patched", False):
        return

    def _drain_and_barrier(self, tick_clock, wait_clock):
        nc = self.nc
        collector = nc.sync.nop(nofuse=True)
        wait_clock.add_sem_waits(
            collector.ins, ScopedClock({None: tick_clock.global_clock})
        )
        si = collector.ins.sync_info
        waits = list(si.on_wait) if si is not None else []
        if len(waits) > 1:
            si.on_wait = waits[:1]
            for w in waits[1:]:
                extra = nc.sync.nop(nofuse=True)
                extra.ins.sync_info = mybir.SyncInfo(on_wait=[w], on_update=[])
        nc.sync.drain()
        nc.all_engine_barrier()
        assert self.sems is not None
        popped = nc._tile_sem_poison_stack.pop()
        assert popped is self._sem_poison
        nc.clear_and_free_semaphores(list(self.sems.allocated().values()))
        nc.all_engine_barrier()

    TileContext._drain_and_barrier = _drain_and_barrier
    TileContext._ant_drain_patched = True


# --------------------------------------------------------------------------
# custom DVE ops: cubic ~ exp(x/4)/C0F (1 pass) and x -> x^4 (1 pass)
_EXP_OPS = {}


def _register_exp_ops():
    if _EXP_OPS:
        return _EXP_OPS
    from concourse import dve_ops
    from concourse.dve_ops import DveOp, OPS, _SUB_OPCODE_FOR_NAME
    from concourse.dve_spec import Spec, Src0, C0, C1, C2, One, sq, lower
    from concourse.dve_uop import DveOpSpec

    def make(name, spec):
        if name in _SUB_OPCODE_FOR_NAME:
            for op in OPS:
                if op.name == name:
                    return op
        row = max(_SUB_OPCODE_FOR_NAME.values()) + 1
        op = DveOp(name, spec, subdim=False, uops_sha={})
        OPS.append(op)
        _SUB_OPCODE_FOR_NAME[name] = row
        dve_ops.CUSTOM_DVE_SPECS[name] = spec
        for ver in ("v3", "v4"):
            uops = lower(spec, ver=ver)
            op.uops_sha[ver] = DveOpSpec(
                name=name, opcode=row, uops=uops, rd1_en=False
            ).sha(ver)
        return op

    cubic = make(
        "EXPC_ANT",
        Spec(
            body=(Src0 * C0 + One) * ((sq(Src0) * C2 + Src0 * C1) + One),
            reference=lambda in0, in1, s0, s1, imm2: (in0 * s0 + 1.0)
            * ((in0 * in0) * imm2 + in0 * s1 + 1.0),
        ),
    )
    pow4 = make(
        "POW4_ANT",
        Spec(
            body=sq(sq(Src0)),
            reference=lambda in0, in1, s0, s1, imm2: (in0 * in0) * (in0 * in0),
        ),
    )
    _EXP_OPS["cubic"] = cubic
    _EXP_OPS["pow4"] = pow4
    return _EXP_OPS


# --------------------------------------------------------------------------
def build_kernel():
    _patch_to_json()
    _patch_tile_drain()
    Exp = mybir.ActivationFunctionType.Exp
    Alu = mybir.AluOpType

    nc = bass.Bass(trn_type="TRN2")
    xT = nc.dram_tensor("xT", [DIM, N], BF16, kind="ExternalInput")
    xT8 = nc.dram_tensor("xT8", [DIM, N], F8, kind="ExternalInput")
    wqk = nc.dram_tensor("wqk", [DIM, 384], F8, kind="ExternalInput")
    bqk = nc.dram_tensor("bqk", [384], F32, kind="ExternalInput")
    dsc = nc.dram_tensor("dsc", [128, 3], F32, kind="ExternalInput")
    wv = nc.dram_tensor("wv", [DIM, 192], BF16, kind="ExternalInput")
    wp = nc.dram_tensor("wp", [192, DIM], BF16, kind="ExternalInput")
    out = nc.dram_tensor("out", [N, DIM], F32, kind="ExternalOutput")

    KC = DIM // 128  # 6 contraction chunks

    with tile.TileContext(nc) as tc:
        with (
            tc.tile_pool(name="persist", bufs=1) as pp,
            tc.tile_pool(name="pt_act", bufs=4) as pta,
            tc.tile_pool(name="scratch", bufs=4) as sp,
            tc.tile_pool(name="osb", bufs=3) as op_,
            tc.tile_pool(name="ysb", bufs=3) as yp,
            tc.tile_pool(name="ps", bufs=3, space="PSUM") as ps,
            tc.tile_pool(name="ps_acc", bufs=2, space="PSUM") as ps_acc,
        ):
            # ---- persistent SBUF ----
            xT_sb = pp.tile([128, KC, N], BF16, tag="xT")
            xT8_sb = pp.tile([128, KC, N], F8, tag="xT8")
            wqk_sb = pp.tile([128, KC, 384], F8, tag="wqk")
            wv_sb = pp.tile([128, KC, 192], BF16, tag="wv")
            wp_sb = pp.tile([128, 2, DIM], BF16, tag="wp")
            bqk_sb = pp.tile([128, 3], F32, tag="bqk")
            dsc_sb = pp.tile([128, 3], F32, tag="dsc")
            actbias_sb = pp.tile([128, 1], F32, tag="actbias")
            warm_sb = pp.tile([128, 8], BF16, tag="warm")
            qk_sb = pp.tile([128, 4, N], BF16, tag="qkT")  # mt: [Q0|Q1],[K0|K1],[Q2|K2],[K2d|Q2d]
            v_sb = pp.tile([128, NKT, 384], F8, tag="vaug")  # per kt: 3x [v_h(64) | ones(64)]

            # PE clock (HAM) warmup on zeroed SBUF + early exp-table load,
            # all before the heavyweight DMAs and memsets are queued.
            warm_in = pp.tile([128, 256], BF16, tag="warmmm")
            nc.gpsimd.memset(warm_in[:], 0.0)
            nc.gpsimd.memset(actbias_sb[:], ACT_BIAS)
            wps = ps.tile([128, 2 * QC], F32, tag="sa", name="warmps")[:, 0:256]
            for i in range(32):
                nc.tensor.matmul(wps[:], warm_in[:, 0:128], warm_in[:],
                                 start=(i == 0), stop=(i == 31))
            nc.scalar.activation(warm_sb[:], actbias_sb[:].to_broadcast((128, 8)), Exp)

            nc.sync.dma_start(wqk_sb[:], wqk.rearrange("(o p) m -> p o m", p=128))
            nc.sync.dma_start(bqk_sb[:], bqk.rearrange("(m p) -> p m", p=128))
            nc.sync.dma_start(dsc_sb[:], dsc[:, :])
            nc.gpsimd.dma_start(wv_sb[:], wv.rearrange("(o p) m -> p o m", p=128))
            nc.gpsimd.dma_start(wp_sb[:, 0, :], wp[0:128, :])
            nc.gpsimd.dma_start(wp_sb[0:64, 1, :], wp[128:192, :])
            # x arrives in (token-chunk, kc) granules so compute starts early:
            # fp8 copy first (feeds qk_phase), bf16 second (feeds v_tile)
            for qq in range(NQC):
                for kc in range(KC):
                    eng = nc.sync if kc % 2 == 0 else nc.gpsimd
                    eng.dma_start(
                        xT8_sb[:, kc, QC * qq : QC * qq + QC],
                        xT8[128 * kc : 128 * kc + 128, QC * qq : QC * qq + QC],
                    )
                for kc in range(KC):
                    eng = nc.gpsimd if kc % 2 == 0 else nc.sync
                    eng.dma_start(
                        xT_sb[:, kc, QC * qq : QC * qq + QC],
                        xT[128 * kc : 128 * kc + 128, QC * qq : QC * qq + QC],
                    )
            nc.vector.memset(v_sb[:], 1.0)

            DR = mybir.MatmulPerfMode.DoubleRow

            def qk_phase(qc):
                # Q^T / K^T projection (fp8 DoubleRow over kc pairs) for one
                # 512-token slice, + head-2 swap copy. The fp8 weight
                # pre-scales are undone in the bias-add (dsc column).
                for mt in range(3):
                    ps_t = ps.tile([128, 2 * QC], F32, tag="sa", name="qkps")[:, 0:QC]
                    for c in range(KC // 2):
                        nc.tensor.matmul(
                            ps_t[:],
                            wqk_sb[:, 2 * c : 2 * c + 2, 128 * mt : 128 * mt + 128],
                            xT8_sb[:, 2 * c : 2 * c + 2, QC * qc : QC * qc + QC],
                            start=(c == 0),
                            stop=(c == KC // 2 - 1),
                            perf_mode=DR,
                        )
                    nc.vector.tensor_scalar(
                        qk_sb[:, mt, QC * qc : QC * qc + QC],
                        ps_t[:],
                        dsc_sb[:, mt : mt + 1],
                        bqk_sb[:, mt : mt + 1],
                        Alu.mult,
                        Alu.add,
                    )
                sl = slice(QC * qc, QC * qc + QC)
                nc.sync.dma_start(qk_sb[0:64, 3, sl], qk_sb[64:128, 2, sl])
                nc.sync.dma_start(qk_sb[64:128, 3, sl], qk_sb[0:64, 2, sl])

            def v_tile(kt):
                    ps_t = ps.tile([128, 2 * QC], F32, tag="sa", name="vps")[:, 0:192]
                    for kc in range(KC):
                        nc.tensor.matmul(
                            ps_t[:],
                            xT_sb[:, kc, KT * kt : KT * kt + KT],
                            wv_sb[:, kc, :],
                            start=(kc == 0),
                            stop=(kc == KC - 1),
                        )
                    nc.vector.tensor_copy(
                        out=v_sb[:, kt, :].rearrange("p (h c) -> p h c", c=128)[:, :, 0:64],
                        in_=ps_t[:].rearrange("p (h c) -> p h c", c=64),
                    )

            # score matmul operands: heads 0/1 pair on partition halves; head 2
            # alternates halves by kt parity via the swapped copy in slot 3.
            def s_operands(h, kt):
                if h < 2:
                    po = 64 * h
                    return (1, po), (0, po)
                return ((3, 0) if kt % 2 == 0 else (2, 64)), ((2, 0) if kt % 2 == 0 else (3, 64))

            def s_mm(dst, h, kt, qc):
                (lm, lp), (rm, rp) = s_operands(h, kt)
                nc.tensor.matmul(
                    dst,
                    qk_sb[lp : lp + 64, lm, KT * kt : KT * kt + KT],
                    qk_sb[rp : rp + 64, rm, QC * qc : QC * qc + QC],
                    start=True,
                    stop=True,
                    tile_position=(lp, 0),
                )

            def attn_begin(qc):
                return {
                    "qc": qc,
                    # oab: cols 0:QC = normalized [h0;h1] o, cols QC:2QC
                    # rows 0:64 = normalized h2 o
                    "oab": op_.tile([128, 2 * QC], BF16, tag="oab", name="oab"),
                    "ocO": sp.tile([128, QC], F32, tag="ocO", name="ocO", bufs=2),
                    "ocS": sp.tile([128, 2 * QC], F32, tag="ocS", name="ocS", bufs=2),
                    "pend": None,
                }

            def _flush(st, keep=0):
                # fp8 DoubleRow PV: kt pair (k0, k0+1) as the two planes.
                # PVs run `keep` supersteps behind their exp so the PE never
                # waits on an exp that was just issued.
                while len(st["pend"]) > keep:
                    _flush1(st, st["pend"].pop(0))

            def _flush1(st, pend):
                kind, k0, pt = pend
                if kind == "01":
                    pr = pt[:].rearrange("p (two x) -> p two x", two=2)
                    for h in (0, 1):
                        nc.tensor.matmul(
                            st["o_ps"][h][:],
                            v_sb[:, k0 : k0 + 2, 128 * h : 128 * h + 128],
                            pr[:, :, QC * h : QC * h + QC],
                            start=(k0 == 0),
                            stop=(k0 == NKT - 2),
                            perf_mode=DR,
                        )
                else:
                    nc.tensor.matmul(
                        st["o_ps"][2][:],
                        v_sb[:, k0 : k0 + 2, 256:384],
                        pt[:].rearrange("p (two x) -> p two x", two=2),
                        start=(k0 == 0),
                        stop=(k0 == NKT - 2),
                        perf_mode=DR,
                    )

            def attn_steps01(st, k0s, dve_b=(), filler=None):
                # heads 0/1, kt pairs; per superstep: 4 score matmuls (2
                # concurrent pairs), 2 exps, 2 DoubleRow PVs (one step late
                # so the PE never waits on the exp it just issued). dve_b
                # routes the kt+1 exp of those supersteps to the DVE
                # schraudolph; filler(j) emits PE filler (proj chunks).
                qc = st["qc"]
                o_ps = st.setdefault("o_ps", {})
                for h in (0, 1):
                    if h not in o_ps:
                        o_ps[h] = ps_acc.tile([128, QC], F32, tag="acc", name="acc")
                for j, k0 in enumerate(k0s):
                    s2a = ps.tile([128, 2 * QC], F32, tag="sa", name="sa")
                    s_mm(s2a[:, 0:QC], 0, k0, qc)
                    s_mm(s2a[:, QC : 2 * QC], 1, k0, qc)
                    s2b = ps.tile([128, 2 * QC], F32, tag="sa", name="sa")
                    s_mm(s2b[:, 0:QC], 0, k0 + 1, qc)
                    s_mm(s2b[:, QC : 2 * QC], 1, k0 + 1, qc)
                    pt = pta.tile([128, 4 * QC], F8, tag="pta", name="pta")
                    if j in dve_b:
                        # whole superstep on the DVE so the two engines never
                        # write the same pta tile concurrently
                        nc.vector.tensor_scalar(
                            pt[:, 0 : 2 * QC].bitcast(mybir.dt.int8), s2a[:],
                            SCH_K, SCH_C, Alu.mult, Alu.add,
                        )
                        nc.vector.tensor_scalar(
                            pt[:, 2 * QC : 4 * QC].bitcast(mybir.dt.int8), s2b[:],
                            SCH_K, SCH_C, Alu.mult, Alu.add,
                        )
                    else:
                        nc.scalar.activation(pt[:, 0 : 2 * QC], s2a[:], Exp, bias=actbias_sb[:])
                        nc.scalar.activation(pt[:, 2 * QC : 4 * QC], s2b[:], Exp, bias=actbias_sb[:])
                    st["pend"].append(("01", k0, pt))
                    _flush(st, keep=1)
                    if filler is not None:
                        filler(j)

            def attn_steps2(st, k0s, filler=None):
                # head 2: even kt on one partition half, odd on the other
                qc = st["qc"]
                o_ps = st.setdefault("o_ps", {})
                if 2 not in o_ps:
                    o_ps[2] = ps_acc.tile([128, QC], F32, tag="acc", name="acc")
                for idx, k0 in enumerate(k0s):
                    s2 = ps.tile([128, 2 * QC], F32, tag="sa", name="sa")
                    s_mm(s2[:, 0:QC], 2, k0, qc)
                    s_mm(s2[:, QC : 2 * QC], 2, k0 + 1, qc)
                    pt = pta.tile([128, 2 * QC], F8, tag="pt2", name="pt2")
                    if idx % 2 == 1:
                        # DVE "exp": schraudolph straight into the fp8 bit
                        # pattern — i8 = round(K*s + C), bitcast e4m3.
                        nc.vector.tensor_scalar(
                            pt[:].bitcast(mybir.dt.int8), s2[:],
                            SCH_K, SCH_C, Alu.mult, Alu.add,
                        )
                    else:
                        nc.scalar.activation(pt[:], s2[:], Exp, bias=actbias_sb[:])
                    st["pend"].append(("2", k0, pt))
                    _flush(st, keep=1)
                    if filler is not None:
                        filler(idx)

            def attn_evac01(st):
                # free heads 0/1 accumulators, assembling (o | sums) staging:
                # ocO = [h0 o ; h1 o], ocS cols 0:QC = [h0 sums ; h1 sums]
                _flush(st)
                o_ps, ocO, ocS = st["o_ps"], st["ocO"], st["ocS"]
                nc.vector.tensor_copy(out=ocO[0:64, :], in_=o_ps[0][0:64, :])
                nc.vector.tensor_copy(out=ocO[64:128, :], in_=o_ps[1][0:64, :])
                nc.vector.tensor_copy(out=ocS[0:64, 0:QC], in_=o_ps[0][64:128, :])
                nc.vector.tensor_copy(out=ocS[64:128, 0:QC], in_=o_ps[1][64:128, :])
                del o_ps[0], o_ps[1]

            def attn_evac2(st):
                # h2 sums into ocS cols QC:2QC (rows 64:128 stay junk); the
                # o half stays in PSUM until the normalize multiply.
                _flush(st)
                nc.vector.tensor_copy(
                    out=st["ocS"][0:64, QC : 2 * QC], in_=st["o_ps"][2][64:128, :]
                )

            MAGIC = 0x7EF311C3

            def normalize(st, last=False):
                # rec = -(approx 1/sums): int bit-trick seed + 1 Newton step on
                # the assembled [128, 2QC] sums tile (3 heads in one chain; the
                # sign is fixed up on the host, partials are negated). The
                # seed/newton chain runs on the Pool engine to keep the DVE
                # free for exp tiles.
                ocS, ocO = st["ocS"], st["ocO"]
                seedt = sp.tile([128, 2 * QC], F32, tag="seed", name="seed", bufs=2)
                nc.vector.tensor_scalar(
                    seedt[:].bitcast(mybir.dt.int32),
                    ocS[:].bitcast(mybir.dt.int32),
                    MAGIC, -1, Alu.subtract, Alu.mult,
                )
                ut = sp.tile([128, 2 * QC], F32, tag="nru", name="nru", bufs=2)
                nc.vector.tensor_tensor(ut[:], ocS[:], seedt[:], Alu.mult)
                rect = sp.tile([128, 2 * QC], F32, tag="recip", name="recip", bufs=2)
                nc.vector.scalar_tensor_tensor(
                    rect[:], ut[:], 2.0, seedt[:], Alu.subtract, Alu.mult
                )
                oab = st["oab"]
                nc.vector.tensor_tensor(oab[:, 0:QC], ocO[:], rect[:, 0:QC], Alu.mult)
                nc.vector.tensor_tensor(
                    oab[0:64, QC : 2 * QC],
                    st["o_ps"][2][0:64, :],
                    rect[0:64, QC : 2 * QC],
                    Alu.mult,
                )
                del st["o_ps"][2]

            def proj_chunk(pst, j):
                # one (qt, column-half) chunk of the output projection for
                # pst's query slice; evictions alternate ScalarE / VectorE.
                qc, oab = pst["qc"], pst["oab"]
                qt, half = divmod(j, 2)
                if half == 0:
                    pst["ys"] = yp.tile([128, DIM], F32, tag="y", name="y")
                ys = pst["ys"]
                nsl = slice(384 * half, 384 * half + 384)
                yps = ps.tile([128, 2 * QC], F32, tag="sa", name="yps")[:, 0:384]
                nc.tensor.matmul(
                    yps[:], oab[:, 128 * qt : 128 * qt + 128], wp_sb[:, 0, nsl],
                    start=True, stop=False,
                )
                nc.tensor.matmul(
                    yps[:],
                    oab[0:64, QC + 128 * qt : QC + 128 * qt + 128],
                    wp_sb[0:64, 1, nsl],
                    start=False, stop=True,
                )
                if half == 0:
                    nc.scalar.copy(ys[:, nsl], yps[:])
                else:
                    nc.vector.tensor_copy(out=ys[:, nsl], in_=yps[:])
                    nc.gpsimd.dma_start(
                        out[QC * qc + 128 * qt : QC * qc + 128 * qt + 128, :], ys[:]
                    )

            # ---- software-pipelined schedule ----
            # Phase 1 (QK/V projections) is interleaved with attention(qc=0):
            # attention consumes K^T/V k-tiles in order, and k-tile group g
            # becomes available right after qk_phase(g)+v_tile(4g..4g+3).
            qk_phase(0)
            for kt in range(0, 4):
                v_tile(kt)
            st0 = attn_begin(0)
            attn_steps01(st0, (0, 2))
            for qq in range(1, NQC):
                qk_phase(qq)
                for kt in range(4 * qq, 4 * qq + 4):
                    v_tile(kt)
                attn_steps01(st0, (4 * qq, 4 * qq + 2))
            attn_evac01(st0)
            attn_steps2(st0, range(0, NKT, 2))
            attn_evac2(st0)
            normalize(st0)
            prev = st0
            for qc in range(1, NQC):
                st = attn_begin(qc)
                attn_steps01(st, range(0, NKT, 2), dve_b=(2, 5))
                attn_evac01(st)
                # proj(qc-1) chunks interleave with the h2 steps: the PE has
                # slack there and the evictions split across ScalarE/VectorE
                attn_steps2(
                    st, range(0, NKT, 2),
                    filler=lambda i, p=prev: proj_chunk(p, i),
                )
                attn_evac2(st)
                normalize(st, last=(qc == NQC - 1))
                prev = st
            for j in range(8):
                proj_chunk(prev, j)
    return nc


_NC_CACHE = {}


def _get_nc():
    if "nc" not in _NC_CACHE:
        _NC_CACHE["nc"] = build_kernel()
    return _NC_CACHE["nc"]


def kernel(x, qkv_w, qkv_b, proj_w, proj_b):
    x = np.asarray(x, np.float32)
    qkv_w = np.asarray(qkv_w, np.float32)
    qkv_b = np.asarray(qkv_b, np.float32)
    proj_w = np.asarray(proj_w, np.float32)
    proj_b = np.asarray(proj_b, np.float32)

    wr = qkv_w.reshape(DIM, 3, H, Dh)
    br = qkv_b.reshape(3, H, Dh)
    scale = Dh ** -0.5

    # fp8 descale vector per mt slot: mt0 all-Q, mt1 all-K, mt2 [Q2|K2]
    dsc_c = np.empty((128, 3), np.float32)
    dsc_c[:, 0] = 1.0 / SQ
    dsc_c[:, 1] = 1.0 / SK
    dsc_c[0:64, 2] = 1.0 / SQ
    dsc_c[64:128, 2] = 1.0 / SK

    in_maps = []
    for core in range(NCORES):
        b, g = divmod(core, 4)
        hs = slice(G * g, G * g + G)
        # fold softmax scale into Q; pre-scale fp8 weights out of subnormals
        wq = wr[:, 0, hs, :].reshape(DIM, G * Dh) * (scale * SQ)
        wk = wr[:, 1, hs, :].reshape(DIM, G * Dh) * SK
        wvm = wr[:, 2, hs, :].reshape(DIM, G * Dh)
        bq = br[0, hs].reshape(G * Dh) * scale
        bk = br[1, hs].reshape(G * Dh)
        # column order: mt0=[Q0|Q1], mt1=[K0|K1], mt2=[Q2|K2] (64 cols per head)
        wqk_c = np.concatenate(
            [wq[:, 0:128], wk[:, 0:128], wq[:, 128:192], wk[:, 128:192]], axis=1
        )
        bqk_c = np.concatenate([bq[0:128], bk[0:128], bq[128:192], bk[128:192]])
        xTb = np.ascontiguousarray(x[b].T)
        in_maps.append(
            {
                "xT": xTb.astype(bf16),
                "xT8": xTb.astype(e4m3).view(np.uint8),
                "wqk": np.ascontiguousarray(wqk_c).astype(e4m3).view(np.uint8),
                "bqk": np.ascontiguousarray(bqk_c),
                "dsc": dsc_c,
                "wv": np.ascontiguousarray(wvm).astype(bf16),
                "wp": np.ascontiguousarray(proj_w[64 * G * g : 64 * G * (g + 1), :]).astype(bf16),
            }
        )

    nc = _get_nc()
    res = run_bass_kernel_spmd(nc, in_maps, core_ids=list(range(NCORES)))
    _NC_CACHE["last_result"] = res

    bias_row = (br[2].reshape(DIM).astype(np.float64) @ proj_w.astype(np.float64)
                + proj_b.astype(np.float64)).astype(np.float32)
    out = np.zeros((B, N, DIM), np.float32)
    for b in range(B):
        acc = np.zeros((N, DIM), np.float64)
        for g in range(4):
            acc += res.results[4 * b + g]["out"].astype(np.float64)
        out[b] = (-acc).astype(np.float32) + bias_row
    return out



# revision 43
# speedup vs baseline: 1.1002x; 1.0215x over previous
"""Trainium2 Bass kernel: multi-head attention (B=2, N=2048, DIM=768, H=12, Dh=64),
sharded (batch x head-group) across 8 NeuronCores. Self-contained.

fp8 fast paths: the QK projection and the PV matmul run as fp8e4m3
DoubleRow matmuls (two contraction planes per instruction, 2 moving
cols/cycle): QK contracts kc pairs of x/w (weights pre-scaled by SQ/SK out
of e4m3's subnormal range, descaled in the on-device bias-add); PV
contracts kt pairs with P emitted in fp8 straight from the ScalarE exp and
V cast to fp8 (ones-columns stay exact). V projection, scores, and the
output projection stay bf16 for accuracy (sim: rel_err 0.015 vs 2e-2 gate).

Per-core shard (core = b*4 + g, g in 0..3, heads 3g..3g+2):
  - computes Q^T,K^T (features on partitions) and V (tokens on partitions) from x[b]^T
  - scores S^T[k,q] per head via row-tiled K=64 matmuls (2 concurrent per slot via
    tile_position partition halves; head 2 pairs its own even/odd k-tiles through a
    partition-swapped copy of Q2/K2)
  - exp on ScalarE over [128,1024] PSUM tiles shared by the head pair
  - O~^T and softmax sums in one matmul: V is augmented with 64 ones-columns so
    rows 64:128 of the accumulator hold the sums broadcast across partitions
  - normalize with a stock-op Newton reciprocal (bit-trick seed; sign fixed on host),
    project with this group's proj_w rows, partial out [2048, 768] f32
Host: shards inputs (bf16, scale folded into Wq, layouts pre-arranged), gathers:
  out[b] = -(sum_g partial_gb) + (qkv_b[v-part] @ proj_w + proj_b).
Scheduling: PE warmup spam against HAM cold-clock, QK/V phase interleaved with the
DMA arrival order and with attention(qc=0); heads 0/1 run as kt-pair supersteps
(4 score matmuls, 2 exps, 2 DoubleRow PVs one superstep behind), head 2 pairs
even/odd kt; a subset of exp tiles goes to the DVE via an int8-schraudolph that
emits e4m3 bit patterns directly; proj(qc-1) chunks interleave with the head-2
steps; normalize runs one Newton chain over all 3 heads' assembled sums.
"""

import sys

for _p in ("/opt/trn_rl_repo",):
    if _p not in sys.path:
        sys.path.append(_p)

import numpy as np
import ml_dtypes

import concourse.bass as bass
import concourse.mybir as mybir
import concourse.tile as tile
from concourse.bass_utils import run_bass_kernel_spmd

BF16 = mybir.dt.bfloat16
F32 = mybir.dt.float32
F8 = mybir.dt.float8e4
bf16 = ml_dtypes.bfloat16
e4m3 = ml_dtypes.float8_e4m3fn

B, N, DIM = 2, 2048, 768
H, Dh = 12, 64
G = 3  # heads per core
NCORES = 8
QC = 512  # query chunk (free dim of score matmuls)
NQC = N // QC
KT = 128  # key tile (partition dim of S^T)
NKT = N // KT

# fp8 weight pre-scales (keep e4m3 operands out of the subnormal range);
# the inverse is applied in the on-device bias-add.
SQ = 2.0**7  # wq (softmax scale folded in, rms ~0.0025)
SK = 2.0**4  # wk (rms ~0.02)

# exp split: which k-tiles go to the VectorE (custom poly) vs ScalarE (table exp).
# DVE k-tiles are singles; ACT k-tiles are grouped in pairs of 2 (one [128,1024] inst).
# NOTE: custom DVE ops fail to encode in this container's walrus ("ISA wrong
# length"), so all exp goes through ScalarE for now.
DVE_KTS = ()
ACT_PAIRS = tuple((2 * i, 2 * i + 1) for i in range(8))

# EXP4 constants: exp(x) ~ C0F^4 * ((1+A x)(1 + B x + CC x^2))^4 on |x| <= 2.75
EXP_A = 0.14770726095997042
EXP_B = 0.10315315610745052
EXP_CC = 0.017226206106509708
EXP_C0F = 0.9990441257079289
ACT_BIAS = -4.0 * float(np.log(EXP_C0F))  # ScalarE computes exp(x + bias) to match

# DVE schraudolph-to-fp8 "exp": i8 = round(K*s + C) bitcast as e4m3 gives
# ~exp(s + ACT_BIAS) with 3.1% rms / 8% max multiplicative error (unbiased,
# so it mixes with exact ACT exp tiles inside one softmax row). Valid for
# s + ACT_BIAS in (-4.5, +5.5); the actual score range here is +-2.4.
SCH_K = 8.0 / float(np.log(2.0))
SCH_C = 56.0 - 0.46 + SCH_K * ACT_BIAS


# --------------------------------------------------------------------------
# workaround: this container's walrus accepts only ONE sync-wait per
# instruction ("Too many sync wait commands"). Split multi-wait sync_infos
# onto same-engine NoOps inserted right before the instruction.
def _patch_to_json():
    import orjson

    if getattr(bass.Bass, "_ant_json_patched", False):
        return
    orig = bass.Bass.to_json_bytes

    def to_json_bytes(self, *a, **kw):
        m = orjson.loads(orig(self, *a, **kw))

        def walk(o):
            if isinstance(o, dict):
                insts = o.get("instructions")
                if isinstance(insts, list) and insts and isinstance(insts[0], dict):
                    new = []
                    for inst in insts:
                        si = inst.get("sync_info")
                        waits = (si or {}).get("on_wait") or []
                        if len(waits) > 1:
                            for i, w in enumerate(waits[:-1]):
                                new.append(
                                    {
                                        "debug": inst.get("debug", 0),
                                        "engine": inst["engine"],
                                        "ins": [],
                                        "name": f"{inst['name']}-sw{i}",
                                        "opcode": "NoOp",
                                        "outs": [],
                                        "sync_info": {
                                            "on_update": [],
                                            "on_wait": [w],
                                        },
                                    }
                                )
                            si["on_wait"] = waits[-1:]
                        new.append(inst)
                    o["instructions"] = new
                for v in o.values():
                    walk(v)
            elif isinstance(o, list):
                for v in o:
                    walk(v)

        walk(m)
        return orjson.dumps(m)

    bass.Bass.to_json_bytes = to_json_bytes
    bass.Bass._ant_json_patched = True


# workaround: this container's walrus allows only 1 sync-wait on SP CTRL ops;
# Tile's kernel-tail drain piles every outstanding proc wait onto one Drain.
def _patch_tile_drain():
    from concourse.tile import TileContext, ScopedClock

    if getattr(TileContext, "_ant_drain_patched", False):
        return

    def _drain_and_barrier(self, tick_clock, wait_clock):
        nc = self.nc
        collector = nc.sync.nop(nofuse=True)
        wait_clock.add_sem_waits(
            collector.ins, ScopedClock({None: tick_clock.global_clock})
        )
        si = collector.ins.sync_info
        waits = list(si.on_wait) if si is not None else []
        if len(waits) > 1:
            si.on_wait = waits[:1]
            for w in waits[1:]:
                extra = nc.sync.nop(nofuse=True)
                extra.ins.sync_info = mybir.SyncInfo(on_wait=[w], on_update=[])
        nc.sync.drain()
        nc.all_engine_barrier()
        assert self.sems is not None
        popped = nc._tile_sem_poison_stack.pop()
        assert popped is self._sem_poison
        nc.clear_and_free_semaphores(list(self.sems.allocated().values()))
        nc.all_engine_barrier()

    TileContext._drain_and_barrier = _drain_and_barrier
    TileContext._ant_drain_patched = True


# --------------------------------------------------------------------------
# custom DVE ops: cubic ~ exp(x/4)/C0F (1 pass) and x -> x^4 (1 pass)
_EXP_OPS = {}


def _register_exp_ops():
    if _EXP_OPS:
        return _EXP_OPS
    from concourse import dve_ops
    from concourse.dve_ops import DveOp, OPS, _SUB_OPCODE_FOR_NAME
    from concourse.dve_spec import Spec, Src0, C0, C1, C2, One, sq, lower
    from concourse.dve_uop import DveOpSpec

    def make(name, spec):
        if name in _SUB_OPCODE_FOR_NAME:
            for op in OPS:
                if op.name == name:
                    return op
        row = max(_SUB_OPCODE_FOR_NAME.values()) + 1
        op = DveOp(name, spec, subdim=False, uops_sha={})
        OPS.append(op)
        _SUB_OPCODE_FOR_NAME[name] = row
        dve_ops.CUSTOM_DVE_SPECS[name] = spec
        for ver in ("v3", "v4"):
            uops = lower(spec, ver=ver)
            op.uops_sha[ver] = DveOpSpec(
                name=name, opcode=row, uops=uops, rd1_en=False
            ).sha(ver)
        return op

    cubic = make(
        "EXPC_ANT",
        Spec(
            body=(Src0 * C0 + One) * ((sq(Src0) * C2 + Src0 * C1) + One),
            reference=lambda in0, in1, s0, s1, imm2: (in0 * s0 + 1.0)
            * ((in0 * in0) * imm2 + in0 * s1 + 1.0),
        ),
    )
    pow4 = make(
        "POW4_ANT",
        Spec(
            body=sq(sq(Src0)),
            reference=lambda in0, in1, s0, s1, imm2: (in0 * in0) * (in0 * in0),
        ),
    )
    _EXP_OPS["cubic"] = cubic
    _EXP_OPS["pow4"] = pow4
    return _EXP_OPS


# --------------------------------------------------------------------------
def build_kernel():
    _patch_to_json()
    _patch_tile_drain()
    Exp = mybir.ActivationFunctionType.Exp
    Alu = mybir.AluOpType

    nc = bass.Bass(trn_type="TRN2")
    xT = nc.dram_tensor("xT", [DIM, N], BF16, kind="ExternalInput")
    xT8 = nc.dram_tensor("xT8", [DIM, N], F8, kind="ExternalInput")
    wqk = nc.dram_tensor("wqk", [DIM, 384], F8, kind="ExternalInput")
    bqk = nc.dram_tensor("bqk", [384], F32, kind="ExternalInput")
    dsc = nc.dram_tensor("dsc", [128, 3], F32, kind="ExternalInput")
    wv = nc.dram_tensor("wv", [DIM, 192], BF16, kind="ExternalInput")
    wp = nc.dram_tensor("wp", [192, DIM], BF16, kind="ExternalInput")
    out = nc.dram_tensor("out", [N, DIM], F32, kind="ExternalOutput")

    KC = DIM // 128  # 6 contraction chunks

    with tile.TileContext(nc) as tc:
        with (
            tc.tile_pool(name="persist", bufs=1) as pp,
            tc.tile_pool(name="pt_act", bufs=4) as pta,
            tc.tile_pool(name="scratch", bufs=4) as sp,
            tc.tile_pool(name="osb", bufs=3) as op_,
            tc.tile_pool(name="ysb", bufs=3) as yp,
            tc.tile_pool(name="ps", bufs=3, space="PSUM") as ps,
            tc.tile_pool(name="ps_acc", bufs=2, space="PSUM") as ps_acc,
        ):
            # ---- persistent SBUF ----
            xT_sb = pp.tile([128, KC, N], BF16, tag="xT")
            xT8_sb = pp.tile([128, KC, N], F8, tag="xT8")
            wqk_sb = pp.tile([128, KC, 384], F8, tag="wqk")
            wv_sb = pp.tile([128, KC, 192], BF16, tag="wv")
            wp_sb = pp.tile([128, 2, DIM], BF16, tag="wp")
            bqk_sb = pp.tile([128, 3], F32, tag="bqk")
            dsc_sb = pp.tile([128, 3], F32, tag="dsc")
            actbias_sb = pp.tile([128, 1], F32, tag="actbias")
            warm_sb = pp.tile([128, 8], BF16, tag="warm")
            qk_sb = pp.tile([128, 4, N], BF16, tag="qkT")  # mt: [Q0|Q1],[K0|K1],[Q2|K2],[K2d|Q2d]
            v_sb = pp.tile([128, NKT, 384], F8, tag="vaug")  # per kt: 3x [v_h(64) | ones(64)]

            # PE clock (HAM) warmup on zeroed SBUF + early exp-table load,
            # all before the heavyweight DMAs and memsets are queued.
            warm_in = pp.tile([128, 256], BF16, tag="warmmm")
            nc.gpsimd.memset(warm_in[:], 0.0)
            nc.gpsimd.memset(actbias_sb[:], ACT_BIAS)
            wps = ps.tile([128, 2 * QC], F32, tag="sa", name="warmps")[:, 0:256]
            for i in range(32):
                nc.tensor.matmul(wps[:], warm_in[:, 0:128], warm_in[:],
                                 start=(i == 0), stop=(i == 31))
            nc.scalar.activation(warm_sb[:], actbias_sb[:].to_broadcast((128, 8)), Exp)

            nc.sync.dma_start(wqk_sb[:], wqk.rearrange("(o p) m -> p o m", p=128))
            nc.sync.dma_start(bqk_sb[:], bqk.rearrange("(m p) -> p m", p=128))
            nc.sync.dma_start(dsc_sb[:], dsc[:, :])
            nc.gpsimd.dma_start(wv_sb[:], wv.rearrange("(o p) m -> p o m", p=128))
            nc.gpsimd.dma_start(wp_sb[:, 0, :], wp[0:128, :])
            nc.gpsimd.dma_start(wp_sb[0:64, 1, :], wp[128:192, :])
            # x arrives in (token-chunk, kc) granules so compute starts early:
            # fp8 copy first (feeds qk_phase), bf16 second (feeds v_tile)
            for qq in range(NQC):
                for kc in range(KC):
                    eng = nc.sync if kc % 2 == 0 else nc.gpsimd
                    eng.dma_start(
                        xT8_sb[:, kc, QC * qq : QC * qq + QC],
                        xT8[128 * kc : 128 * kc + 128, QC * qq : QC * qq + QC],
                    )
                for kc in range(KC):
                    eng = nc.gpsimd if kc % 2 == 0 else nc.sync
                    eng.dma_start(
                        xT_sb[:, kc, QC * qq : QC * qq + QC],
                        xT[128 * kc : 128 * kc + 128, QC * qq : QC * qq + QC],
                    )
            nc.vector.memset(v_sb[:], 1.0)

            DR = mybir.MatmulPerfMode.DoubleRow

            def qk_phase(qc):
                # Q^T / K^T projection (fp8 DoubleRow over kc pairs) for one
                # 512-token slice, + head-2 swap copy. The fp8 weight
                # pre-scales are undone in the bias-add (dsc column).
                for mt in range(3):
                    ps_t = ps.tile([128, 2 * QC], F32, tag="sa", name="qkps")[:, 0:QC]
                    for c in range(KC // 2):
                        nc.tensor.matmul(
                            ps_t[:],
                            wqk_sb[:, 2 * c : 2 * c + 2, 128 * mt : 128 * mt + 128],
                            xT8_sb[:, 2 * c : 2 * c + 2, QC * qc : QC * qc + QC],
                            start=(c == 0),
                            stop=(c == KC // 2 - 1),
                            perf_mode=DR,
                        )
                    nc.vector.tensor_scalar(
                        qk_sb[:, mt, QC * qc : QC * qc + QC],
                        ps_t[:],
                        dsc_sb[:, mt : mt + 1],
                        bqk_sb[:, mt : mt + 1],
                        Alu.mult,
                        Alu.add,
                    )
                sl = slice(QC * qc, QC * qc + QC)
                nc.sync.dma_start(qk_sb[0:64, 3, sl], qk_sb[64:128, 2, sl])
                nc.sync.dma_start(qk_sb[64:128, 3, sl], qk_sb[0:64, 2, sl])

            def v_tile(kt):
                    ps_t = ps.tile([128, 2 * QC], F32, tag="sa", name="vps")[:, 0:192]
                    for kc in range(KC):
                        nc.tensor.matmul(
                            ps_t[:],
                            xT_sb[:, kc, KT * kt : KT * kt + KT],
                            wv_sb[:, kc, :],
                            start=(kc == 0),
                            stop=(kc == KC - 1),
                        )
                    nc.vector.tensor_copy(
                        out=v_sb[:, kt, :].rearrange("p (h c) -> p h c", c=128)[:, :, 0:64],
                        in_=ps_t[:].rearrange("p (h c) -> p h c", c=64),
                    )

            # score matmul operands: heads 0/1 pair on partition halves; head 2
            # alternates halves by kt parity via the swapped copy in slot 3.
            def s_operands(h, kt):
                if h < 2:
                    po = 64 * h
                    return (1, po), (0, po)
                return ((3, 0) if kt % 2 == 0 else (2, 64)), ((2, 0) if kt % 2 == 0 else (3, 64))

            def s_mm(dst, h, kt, qc):
                (lm, lp), (rm, rp) = s_operands(h, kt)
                nc.tensor.matmul(
                    dst,
                    qk_sb[lp : lp + 64, lm, KT * kt : KT * kt + KT],
                    qk_sb[rp : rp + 64, rm, QC * qc : QC * qc + QC],
                    start=True,
                    stop=True,
                    tile_position=(lp, 0),
                )

            def attn_begin(qc):
                return {
                    "qc": qc,
                    # oab: cols 0:QC = normalized [h0;h1] o, cols QC:2QC
                    # rows 0:64 = normalized h2 o
                    "oab": op_.tile([128, 2 * QC], BF16, tag="oab", name="oab"),
                    "ocO": sp.tile([128, QC], F32, tag="ocO", name="ocO", bufs=2),
                    "ocS": sp.tile([128, 2 * QC], F32, tag="ocS", name="ocS", bufs=2),
                    "pend": None,
                }

            def _flush(st, keep=0):
                # fp8 DoubleRow PV: kt pair (k0, k0+1) as the two planes.
                # PVs run `keep` supersteps behind their exp so the PE never
                # waits on an exp that was just issued.
                while len(st["pend"]) > keep:
                    _flush1(st, st["pend"].pop(0))

            def _flush1(st, pend):
                kind, k0, pt = pend
                if kind == "01":
                    pr = pt[:].rearrange("p (two x) -> p two x", two=2)
                    for h in (0, 1):
                        nc.tensor.matmul(
                            st["o_ps"][h][:],
                            v_sb[:, k0 : k0 + 2, 128 * h : 128 * h + 128],
                            pr[:, :, QC * h : QC * h + QC],
                            start=(k0 == 0),
                            stop=(k0 == NKT - 2),
                            perf_mode=DR,
                        )
                else:
                    nc.tensor.matmul(
                        st["o_ps"][2][:],
                        v_sb[:, k0 : k0 + 2, 256:384],
                        pt[:].rearrange("p (two x) -> p two x", two=2),
                        start=(k0 == 0),
                        stop=(k0 == NKT - 2),
                        perf_mode=DR,
                    )

            def attn_steps01(st, k0s, dve_b=(), filler=None):
                # heads 0/1, kt pairs; per superstep: 4 score matmuls (2
                # concurrent pairs), 2 exps, 2 DoubleRow PVs (one step late
                # so the PE never waits on the exp it just issued). dve_b
                # routes the kt+1 exp of those supersteps to the DVE
                # schraudolph; filler(j) emits PE filler (proj chunks).
                qc = st["qc"]
                o_ps = st.setdefault("o_ps", {})
                for h in (0, 1):
                    if h not in o_ps:
                        o_ps[h] = ps_acc.tile([128, QC], F32, tag="acc", name="acc")
                for j, k0 in enumerate(k0s):
                    s2a = ps.tile([128, 2 * QC], F32, tag="sa", name="sa")
                    s_mm(s2a[:, 0:QC], 0, k0, qc)
                    s_mm(s2a[:, QC : 2 * QC], 1, k0, qc)
                    s2b = ps.tile([128, 2 * QC], F32, tag="sa", name="sa")
                    s_mm(s2b[:, 0:QC], 0, k0 + 1, qc)
                    s_mm(s2b[:, QC : 2 * QC], 1, k0 + 1, qc)
                    pt = pta.tile([128, 4 * QC], F8, tag="pta", name="pta")
                    if j in dve_b:
                        # whole superstep on the DVE so the two engines never
                        # write the same pta tile concurrently
                        nc.vector.tensor_scalar(
                            pt[:, 0 : 2 * QC].bitcast(mybir.dt.int8), s2a[:],
                            SCH_K, SCH_C, Alu.mult, Alu.add,
                        )
                        nc.vector.tensor_scalar(
                            pt[:, 2 * QC : 4 * QC].bitcast(mybir.dt.int8), s2b[:],
                            SCH_K, SCH_C, Alu.mult, Alu.add,
                        )
                    else:
                        nc.scalar.activation(pt[:, 0 : 2 * QC], s2a[:], Exp, bias=actbias_sb[:])
                        nc.scalar.activation(pt[:, 2 * QC : 4 * QC], s2b[:], Exp, bias=actbias_sb[:])
                    st["pend"].append(("01", k0, pt))
                    _flush(st, keep=1)
                    if filler is not None:
                        filler(j)

            def attn_steps2(st, k0s, filler=None):
                # head 2: even kt on one partition half, odd on the other
                qc = st["qc"]
                o_ps = st.setdefault("o_ps", {})
                if 2 not in o_ps:
                    o_ps[2] = ps_acc.tile([128, QC], F32, tag="acc", name="acc")
                for idx, k0 in enumerate(k0s):
                    s2 = ps.tile([128, 2 * QC], F32, tag="sa", name="sa")
                    s_mm(s2[:, 0:QC], 2, k0, qc)
                    s_mm(s2[:, QC : 2 * QC], 2, k0 + 1, qc)
                    pt = pta.tile([128, 2 * QC], F8, tag="pt2", name="pt2")
                    if idx % 2 == 1:
                        # DVE "exp": schraudolph straight into the fp8 bit
                        # pattern — i8 = round(K*s + C), bitcast e4m3.
                        nc.vector.tensor_scalar(
                            pt[:].bitcast(mybir.dt.int8), s2[:],
                            SCH_K, SCH_C, Alu.mult, Alu.add,
                        )
                    else:
                        nc.scalar.activation(pt[:], s2[:], Exp, bias=actbias_sb[:])
                    st["pend"].append(("2", k0, pt))
                    _flush(st, keep=1)
                    if filler is not None:
                        filler(idx)

            def attn_evac01(st):
                # free heads 0/1 accumulators, assembling (o | sums) staging:
                # ocO = [h0 o ; h1 o], ocS cols 0:QC = [h0 sums ; h1 sums]
                _flush(st)
                o_ps, ocO, ocS = st["o_ps"], st["ocO"], st["ocS"]
                nc.vector.tensor_copy(out=ocO[0:64, :], in_=o_ps[0][0:64, :])
                nc.vector.tensor_copy(out=ocO[64:128, :], in_=o_ps[1][0:64, :])
                nc.vector.tensor_copy(out=ocS[0:64, 0:QC], in_=o_ps[0][64:128, :])
                nc.vector.tensor_copy(out=ocS[64:128, 0:QC], in_=o_ps[1][64:128, :])
                del o_ps[0], o_ps[1]

            def attn_evac2(st):
                # h2 sums into ocS cols QC:2QC (rows 64:128 stay junk); the
                # o half stays in PSUM until the normalize multiply.
                _flush(st)
                nc.vector.tensor_copy(
                    out=st["ocS"][0:64, QC : 2 * QC], in_=st["o_ps"][2][64:128, :]
                )

            MAGIC = 0x7EF311C3

            def normalize(st, last=False):
                # rec = -(approx 1/sums): int bit-trick seed + 1 Newton step on
                # the assembled [128, 2QC] sums tile (3 heads in one chain; the
                # sign is fixed up on the host, partials are negated). The
                # seed/newton chain runs on the Pool engine to keep the DVE
                # free for exp tiles.
                ocS, ocO = st["ocS"], st["ocO"]
                seedt = sp.tile([128, 2 * QC], F32, tag="seed", name="seed", bufs=2)
                nc.vector.tensor_scalar(
                    seedt[:].bitcast(mybir.dt.int32),
                    ocS[:].bitcast(mybir.dt.int32),
                    MAGIC, -1, Alu.subtract, Alu.mult,
                )
                ut = sp.tile([128, 2 * QC], F32, tag="nru", name="nru", bufs=2)
                nc.vector.tensor_tensor(ut[:], ocS[:], seedt[:], Alu.mult)
                rect = sp.tile([128, 2 * QC], F32, tag="recip", name="recip", bufs=2)
                nc.vector.scalar_tensor_tensor(
                    rect[:], ut[:], 2.0, seedt[:], Alu.subtract, Alu.mult
                )
                oab = st["oab"]
                nc.vector.tensor_tensor(oab[:, 0:QC], ocO[:], rect[:, 0:QC], Alu.mult)
                nc.vector.tensor_tensor(
                    oab[0:64, QC : 2 * QC],
                    st["o_ps"][2][0:64, :],
                    rect[0:64, QC : 2 * QC],
                    Alu.mult,
                )
                del st["o_ps"][2]

            def proj_chunk(pst, j):
                # one (qt, column-half) chunk of the output projection for
                # pst's query slice; evictions alternate ScalarE / VectorE.
                qc, oab = pst["qc"], pst["oab"]
                qt, half = divmod(j, 2)
                if half == 0:
                    pst["ys"] = yp.tile([128, DIM], F32, tag="y", name="y")
                ys = pst["ys"]
                nsl = slice(384 * half, 384 * half + 384)
                yps = ps.tile([128, 2 * QC], F32, tag="sa", name="yps")[:, 0:384]
                nc.tensor.matmul(
                    yps[:], oab[:, 128 * qt : 128 * qt + 128], wp_sb[:, 0, nsl],
                    start=True, stop=False,
                )
                nc.tensor.matmul(
                    yps[:],
                    oab[0:64, QC + 128 * qt : QC + 128 * qt + 128],
                    wp_sb[0:64, 1, nsl],
                    start=False, stop=True,
                )
                if half == 0:
                    nc.scalar.copy(ys[:, nsl], yps[:])
                else:
                    nc.vector.tensor_copy(out=ys[:, nsl], in_=yps[:])
                    nc.gpsimd.dma_start(
                        out[QC * qc + 128 * qt : QC * qc + 128 * qt + 128, :], ys[:]
                    )

            # ---- software-pipelined schedule ----
            # Phase 1 (QK/V projections) is interleaved with attention(qc=0):
            # attention consumes K^T/V k-tiles in order, and k-tile group g
            # becomes available right after qk_phase(g)+v_tile(4g..4g+3).
            qk_phase(0)
            for kt in range(0, 4):
                v_tile(kt)
            st0 = attn_begin(0)
            attn_steps01(st0, (0, 2))
            for qq in range(1, NQC):
                qk_phase(qq)
                for kt in range(4 * qq, 4 * qq + 4):
                    v_tile(kt)
                attn_steps01(st0, (4 * qq, 4 * qq + 2))
            attn_evac01(st0)
            attn_steps2(st0, range(0, NKT, 2))
            attn_evac2(st0)
            normalize(st0)
            prev = st0
            for qc in range(1, NQC):
                st = attn_begin(qc)
                attn_steps01(st, range(0, NKT, 2), dve_b=(2, 5))
                attn_evac01(st)
                # proj(qc-1) chunks interleave with the h2 steps: the PE has
                # slack there and the evictions split across ScalarE/VectorE
                attn_steps2(
                    st, range(0, NKT, 2),
                    filler=lambda i, p=prev: proj_chunk(p, i),
                )
                attn_evac2(st)
                normalize(st, last=(qc == NQC - 1))
                prev = st
            for j in range(8):
                proj_chunk(prev, j)
    return nc


_NC_CACHE = {}


def _get_nc():
    if "nc" not in _NC_CACHE:
        _NC_CACHE["nc"] = build_kernel()
    return _NC_CACHE["nc"]


def kernel(x, qkv_w, qkv_b, proj_w, proj_b):
    x = np.asarray(x, np.float32)
    qkv_w = np.asarray(qkv_w, np.float32)
    qkv_b = np.asarray(qkv_b, np.float32)
    proj_w = np.asarray(proj_w, np.float32)
    proj_b = np.asarray(proj_b, np.float32)

    wr = qkv_w.reshape(DIM, 3, H, Dh)
    br = qkv_b.reshape(3, H, Dh)
    scale = Dh ** -0.5

    # fp8 descale vector per mt slot: mt0 all-Q, mt1 all-K, mt2 [Q2|K2]
    dsc_c = np.empty((128, 3), np.float32)
    dsc_c[:, 0] = 1.0 / SQ
    dsc_c[:, 1] = 1.0 / SK
    dsc_c[0:64, 2] = 1.0 / SQ
    dsc_c[64:128, 2] = 1.0 / SK

    in_maps = []
    for core in range(NCORES):
        b, g = divmod(core, 4)
        hs = slice(G * g, G * g + G)
        # fold softmax scale into Q; pre-scale fp8 weights out of subnormals
        wq = wr[:, 0, hs, :].reshape(DIM, G * Dh) * (scale * SQ)
        wk = wr[:, 1, hs, :].reshape(DIM, G * Dh) * SK
        wvm = wr[:, 2, hs, :].reshape(DIM, G * Dh)
        bq = br[0, hs].reshape(G * Dh) * scale
        bk = br[1, hs].reshape(G * Dh)
        # column order: mt0=[Q0|Q1], mt1=[K0|K1], mt2=[Q2|K2] (64 cols per head)
        wqk_c = np.concatenate(
            [wq[:, 0:128], wk[:, 0:128], wq[:, 128:192], wk[:, 128:192]], axis=1
        )
        bqk_c = np.concatenate([bq[0:128], bk[0:128], bq[128:192], bk[128:192]])
        xTb = np.ascontiguousarray(x[b].T)
        in_maps.append(
            {
                "xT": xTb.astype(bf16),
                "xT8": xTb.astype(e4m3).view(np.uint8),
                "wqk": np.ascontiguousarray(wqk_c).astype(e4m3).view(np.uint8),
                "bqk": np.ascontiguousarray(bqk_c),
                "dsc": dsc_c,
                "wv": np.ascontiguousarray(wvm).astype(bf16),
                "wp": np.ascontiguousarray(proj_w[64 * G * g : 64 * G * (g + 1), :]).astype(bf16),
            }
        )

    nc = _get_nc()
    res = run_bass_kernel_spmd(nc, in_maps, core_ids=list(range(NCORES)))
    _NC_CACHE["last_result"] = res

    bias_row = (br[2].reshape(DIM).astype(np.float64) @ proj_w.astype(np.float64)
                + proj_b.astype(np.float64)).astype(np.float32)
    out = np.zeros((B, N, DIM), np.float32)
    for b in range(B):
        acc = np.zeros((N, DIM), np.float64)
        for g in range(4):
            acc += res.results[4 * b + g]["out"].astype(np.float64)
        out[b] = (-acc).astype(np.float32) + bias_row
    return out



# revision 45
# speedup vs baseline: 1.1371x; 1.0335x over previous
"""Trainium2 Bass kernel: multi-head attention (B=2, N=2048, DIM=768, H=12, Dh=64),
sharded (batch x head-group) across 8 NeuronCores. Self-contained.

fp8 fast paths: the QK projection and the PV matmul run as fp8e4m3
DoubleRow matmuls (two contraction planes per instruction, 2 moving
cols/cycle): QK contracts kc pairs of x/w (weights pre-scaled by SQ/SK out
of e4m3's subnormal range, descaled in the on-device bias-add); PV
contracts kt pairs with P emitted in fp8 straight from the ScalarE exp and
V cast to fp8 (ones-columns stay exact). V projection, scores, and the
output projection stay bf16 for accuracy (sim: rel_err 0.015 vs 2e-2 gate).

Per-core shard (core = b*4 + g, g in 0..3, heads 3g..3g+2):
  - computes Q^T,K^T (features on partitions) and V (tokens on partitions) from x[b]^T
  - scores S^T[k,q] per head via row-tiled K=64 matmuls (2 concurrent per slot via
    tile_position partition halves; head 2 pairs its own even/odd k-tiles through a
    partition-swapped copy of Q2/K2)
  - exp on ScalarE over [128,1024] PSUM tiles shared by the head pair
  - O~^T and softmax sums in one matmul: V is augmented with 64 ones-columns so
    rows 64:128 of the accumulator hold the sums broadcast across partitions
  - normalize with a stock-op Newton reciprocal (bit-trick seed; sign fixed on host),
    project with this group's proj_w rows, partial out [2048, 768] f32
Host: shards inputs (bf16, scale folded into Wq, layouts pre-arranged), gathers:
  out[b] = -(sum_g partial_gb) + (qkv_b[v-part] @ proj_w + proj_b).
Scheduling: PE warmup spam against HAM cold-clock, QK/V phase interleaved with the
DMA arrival order and with attention(qc=0); heads 0/1 run as kt-pair supersteps
(4 score matmuls, 2 exps, 2 DoubleRow PVs one superstep behind), head 2 pairs
even/odd kt; a subset of exp tiles goes to the DVE via an int8-schraudolph that
emits e4m3 bit patterns directly; proj(qc-1) chunks interleave with the head-2
steps; normalize runs one Newton chain over all 3 heads' assembled sums.
"""

import sys

for _p in ("/opt/trn_rl_repo",):
    if _p not in sys.path:
        sys.path.append(_p)

import numpy as np
import ml_dtypes

import concourse.bass as bass
import concourse.mybir as mybir
import concourse.tile as tile
from concourse.bass_utils import run_bass_kernel_spmd

BF16 = mybir.dt.bfloat16
F32 = mybir.dt.float32
F8 = mybir.dt.float8e4
bf16 = ml_dtypes.bfloat16
e4m3 = ml_dtypes.float8_e4m3fn

B, N, DIM = 2, 2048, 768
H, Dh = 12, 64
G = 3  # heads per core
NCORES = 8
QC = 512  # query chunk (free dim of score matmuls)
NQC = N // QC
KT = 128  # key tile (partition dim of S^T)
NKT = N // KT

# fp8 weight pre-scales (keep e4m3 operands out of the subnormal range);
# the inverse is applied in the on-device bias-add.
SQ = 2.0**7  # wq (softmax scale folded in, rms ~0.0025)
SK = 2.0**4  # wk (rms ~0.02)

# exp split: which k-tiles go to the VectorE (custom poly) vs ScalarE (table exp).
# DVE k-tiles are singles; ACT k-tiles are grouped in pairs of 2 (one [128,1024] inst).
# NOTE: custom DVE ops fail to encode in this container's walrus ("ISA wrong
# length"), so all exp goes through ScalarE for now.
DVE_KTS = ()
ACT_PAIRS = tuple((2 * i, 2 * i + 1) for i in range(8))

# EXP4 constants: exp(x) ~ C0F^4 * ((1+A x)(1 + B x + CC x^2))^4 on |x| <= 2.75
EXP_A = 0.14770726095997042
EXP_B = 0.10315315610745052
EXP_CC = 0.017226206106509708
EXP_C0F = 0.9990441257079289
ACT_BIAS = -4.0 * float(np.log(EXP_C0F))  # ScalarE computes exp(x + bias) to match

# DVE schraudolph-to-fp8 "exp": i8 = round(K*s + C) bitcast as e4m3 gives
# ~exp(s + ACT_BIAS) with 3.1% rms / 8% max multiplicative error (unbiased,
# so it mixes with exact ACT exp tiles inside one softmax row). Valid for
# s + ACT_BIAS in (-4.5, +5.5); the actual score range here is +-2.4.
SCH_K = 8.0 / float(np.log(2.0))
SCH_C = 56.0 - 0.46 + SCH_K * ACT_BIAS


# --------------------------------------------------------------------------
# workaround: this container's walrus accepts only ONE sync-wait per
# instruction ("Too many sync wait commands"). Split multi-wait sync_infos
# onto same-engine NoOps inserted right before the instruction.
def _patch_to_json():
    import orjson

    if getattr(bass.Bass, "_ant_json_patched", False):
        return
    orig = bass.Bass.to_json_bytes

    def to_json_bytes(self, *a, **kw):
        m = orjson.loads(orig(self, *a, **kw))

        def walk(o):
            if isinstance(o, dict):
                insts = o.get("instructions")
                if isinstance(insts, list) and insts and isinstance(insts[0], dict):
                    new = []
                    for inst in insts:
                        si = inst.get("sync_info")
                        waits = (si or {}).get("on_wait") or []
                        if len(waits) > 1:
                            for i, w in enumerate(waits[:-1]):
                                new.append(
                                    {
                                        "debug": inst.get("debug", 0),
                                        "engine": inst["engine"],
                                        "ins": [],
                                        "name": f"{inst['name']}-sw{i}",
                                        "opcode": "NoOp",
                                        "outs": [],
                                        "sync_info": {
                                            "on_update": [],
                                            "on_wait": [w],
                                        },
                                    }
                                )
                            si["on_wait"] = waits[-1:]
                        new.append(inst)
                    o["instructions"] = new
                for v in o.values():
                    walk(v)
            elif isinstance(o, list):
                for v in o:
                    walk(v)

        walk(m)
        return orjson.dumps(m)

    bass.Bass.to_json_bytes = to_json_bytes
    bass.Bass._ant_json_patched = True


# workaround: this container's walrus allows only 1 sync-wait on SP CTRL ops;
# Tile's kernel-tail drain piles every outstanding proc wait onto one Drain.
def _patch_tile_drain():
    from concourse.tile import TileContext, ScopedClock

    if getattr(TileContext, "_ant_drain_patched", False):
        return

    def _drain_and_barrier(self, tick_clock, wait_clock):
        nc = self.nc
        collector = nc.sync.nop(nofuse=True)
        wait_clock.add_sem_waits(
            collector.ins, ScopedClock({None: tick_clock.global_clock})
        )
        si = collector.ins.sync_info
        waits = list(si.on_wait) if si is not None else []
        if len(waits) > 1:
            si.on_wait = waits[:1]
            for w in waits[1:]:
                extra = nc.sync.nop(nofuse=True)
                extra.ins.sync_info = mybir.SyncInfo(on_wait=[w], on_update=[])
        nc.sync.drain()
        nc.all_engine_barrier()
        assert self.sems is not None
        popped = nc._tile_sem_poison_stack.pop()
        assert popped is self._sem_poison
        nc.clear_and_free_semaphores(list(self.sems.allocated().values()))
        nc.all_engine_barrier()

    TileContext._drain_and_barrier = _drain_and_barrier
    TileContext._ant_drain_patched = True


# --------------------------------------------------------------------------
# custom DVE ops: cubic ~ exp(x/4)/C0F (1 pass) and x -> x^4 (1 pass)
_EXP_OPS = {}


def _register_exp_ops():
    if _EXP_OPS:
        return _EXP_OPS
    from concourse import dve_ops
    from concourse.dve_ops import DveOp, OPS, _SUB_OPCODE_FOR_NAME
    from concourse.dve_spec import Spec, Src0, C0, C1, C2, One, sq, lower
    from concourse.dve_uop import DveOpSpec

    def make(name, spec):
        if name in _SUB_OPCODE_FOR_NAME:
            for op in OPS:
                if op.name == name:
                    return op
        row = max(_SUB_OPCODE_FOR_NAME.values()) + 1
        op = DveOp(name, spec, subdim=False, uops_sha={})
        OPS.append(op)
        _SUB_OPCODE_FOR_NAME[name] = row
        dve_ops.CUSTOM_DVE_SPECS[name] = spec
        for ver in ("v3", "v4"):
            uops = lower(spec, ver=ver)
            op.uops_sha[ver] = DveOpSpec(
                name=name, opcode=row, uops=uops, rd1_en=False
            ).sha(ver)
        return op

    cubic = make(
        "EXPC_ANT",
        Spec(
            body=(Src0 * C0 + One) * ((sq(Src0) * C2 + Src0 * C1) + One),
            reference=lambda in0, in1, s0, s1, imm2: (in0 * s0 + 1.0)
            * ((in0 * in0) * imm2 + in0 * s1 + 1.0),
        ),
    )
    pow4 = make(
        "POW4_ANT",
        Spec(
            body=sq(sq(Src0)),
            reference=lambda in0, in1, s0, s1, imm2: (in0 * in0) * (in0 * in0),
        ),
    )
    _EXP_OPS["cubic"] = cubic
    _EXP_OPS["pow4"] = pow4
    return _EXP_OPS


# --------------------------------------------------------------------------
def build_kernel():
    _patch_to_json()
    _patch_tile_drain()
    Exp = mybir.ActivationFunctionType.Exp
    Alu = mybir.AluOpType

    nc = bass.Bass(trn_type="TRN2")
    xT = nc.dram_tensor("xT", [DIM, N], BF16, kind="ExternalInput")
    xT8 = nc.dram_tensor("xT8", [DIM, N], F8, kind="ExternalInput")
    wqk = nc.dram_tensor("wqk", [DIM, 384], F8, kind="ExternalInput")
    bqk = nc.dram_tensor("bqk", [384], F32, kind="ExternalInput")
    dsc = nc.dram_tensor("dsc", [128, 3], F32, kind="ExternalInput")
    wv = nc.dram_tensor("wv", [DIM, 192], BF16, kind="ExternalInput")
    wp = nc.dram_tensor("wp", [192, DIM], BF16, kind="ExternalInput")
    out = nc.dram_tensor("out", [N, DIM], F32, kind="ExternalOutput")

    KC = DIM // 128  # 6 contraction chunks

    with tile.TileContext(nc) as tc:
        with (
            tc.tile_pool(name="persist", bufs=1) as pp,
            tc.tile_pool(name="pt_act", bufs=4) as pta,
            tc.tile_pool(name="scratch", bufs=4) as sp,
            tc.tile_pool(name="osb", bufs=3) as op_,
            tc.tile_pool(name="ysb", bufs=3) as yp,
            tc.tile_pool(name="ps", bufs=3, space="PSUM") as ps,
            tc.tile_pool(name="ps_acc", bufs=2, space="PSUM") as ps_acc,
        ):
            # ---- persistent SBUF ----
            xT_sb = pp.tile([128, KC, N], BF16, tag="xT")
            xT8_sb = pp.tile([128, KC, N], F8, tag="xT8")
            wqk_sb = pp.tile([128, KC, 384], F8, tag="wqk")
            wv_sb = pp.tile([128, KC, 192], BF16, tag="wv")
            wp_sb = pp.tile([128, 2, DIM], BF16, tag="wp")
            bqk_sb = pp.tile([128, 3], F32, tag="bqk")
            dsc_sb = pp.tile([128, 3], F32, tag="dsc")
            actbias_sb = pp.tile([128, 1], F32, tag="actbias")
            warm_sb = pp.tile([128, 8], BF16, tag="warm")
            qk_sb = pp.tile([128, 4, N], BF16, tag="qkT")  # mt: [Q0|Q1],[K0|K1],[Q2|K2],[K2d|Q2d]
            v_sb = pp.tile([128, NKT, 384], F8, tag="vaug")  # per kt: 3x [v_h(64) | ones(64)]

            # PE clock (HAM) warmup on zeroed SBUF + early exp-table load,
            # all before the heavyweight DMAs and memsets are queued.
            warm_in = pp.tile([128, 256], BF16, tag="warmmm")
            nc.gpsimd.memset(warm_in[:], 0.0)
            nc.gpsimd.memset(actbias_sb[:], ACT_BIAS)
            wps = ps.tile([128, 2 * QC], F32, tag="sa", name="warmps")[:, 0:256]
            for i in range(32):
                nc.tensor.matmul(wps[:], warm_in[:, 0:128], warm_in[:],
                                 start=(i == 0), stop=(i == 31))
            nc.scalar.activation(warm_sb[:], actbias_sb[:].to_broadcast((128, 8)), Exp)

            nc.sync.dma_start(wqk_sb[:], wqk.rearrange("(o p) m -> p o m", p=128))
            nc.sync.dma_start(bqk_sb[:], bqk.rearrange("(m p) -> p m", p=128))
            nc.sync.dma_start(dsc_sb[:], dsc[:, :])
            nc.gpsimd.dma_start(wv_sb[:], wv.rearrange("(o p) m -> p o m", p=128))
            nc.gpsimd.dma_start(wp_sb[:, 0, :], wp[0:128, :])
            nc.gpsimd.dma_start(wp_sb[0:64, 1, :], wp[128:192, :])
            # x arrives in (token-chunk, kc) granules so compute starts early:
            # fp8 copy first (feeds qk_phase), bf16 second (feeds v_tile)
            for qq in range(NQC):
                for kc in range(KC):
                    eng = nc.sync if kc % 2 == 0 else nc.gpsimd
                    eng.dma_start(
                        xT8_sb[:, kc, QC * qq : QC * qq + QC],
                        xT8[128 * kc : 128 * kc + 128, QC * qq : QC * qq + QC],
                    )
                for kc in range(KC):
                    eng = nc.gpsimd if kc % 2 == 0 else nc.sync
                    eng.dma_start(
                        xT_sb[:, kc, QC * qq : QC * qq + QC],
                        xT[128 * kc : 128 * kc + 128, QC * qq : QC * qq + QC],
                    )
            nc.vector.memset(v_sb[:], 1.0)

            DR = mybir.MatmulPerfMode.DoubleRow

            def qk_phase(qc):
                # Q^T / K^T projection (fp8 DoubleRow over kc pairs) for one
                # 512-token slice, + head-2 swap copy. The fp8 weight
                # pre-scales are undone in the bias-add (dsc column).
                for mt in range(3):
                    ps_t = ps.tile([128, 2 * QC], F32, tag="sa", name="qkps")[:, 0:QC]
                    for c in range(KC // 2):
                        nc.tensor.matmul(
                            ps_t[:],
                            wqk_sb[:, 2 * c : 2 * c + 2, 128 * mt : 128 * mt + 128],
                            xT8_sb[:, 2 * c : 2 * c + 2, QC * qc : QC * qc + QC],
                            start=(c == 0),
                            stop=(c == KC // 2 - 1),
                            perf_mode=DR,
                        )
                    nc.vector.tensor_scalar(
                        qk_sb[:, mt, QC * qc : QC * qc + QC],
                        ps_t[:],
                        dsc_sb[:, mt : mt + 1],
                        bqk_sb[:, mt : mt + 1],
                        Alu.mult,
                        Alu.add,
                    )
                sl = slice(QC * qc, QC * qc + QC)
                nc.sync.dma_start(qk_sb[0:64, 3, sl], qk_sb[64:128, 2, sl])
                nc.sync.dma_start(qk_sb[64:128, 3, sl], qk_sb[0:64, 2, sl])

            def v_tile(kt):
                    ps_t = ps.tile([128, 2 * QC], F32, tag="sa", name="vps")[:, 0:192]
                    for kc in range(KC):
                        nc.tensor.matmul(
                            ps_t[:],
                            xT_sb[:, kc, KT * kt : KT * kt + KT],
                            wv_sb[:, kc, :],
                            start=(kc == 0),
                            stop=(kc == KC - 1),
                        )
                    nc.vector.tensor_copy(
                        out=v_sb[:, kt, :].rearrange("p (h c) -> p h c", c=128)[:, :, 0:64],
                        in_=ps_t[:].rearrange("p (h c) -> p h c", c=64),
                    )

            # score matmul operands: heads 0/1 pair on partition halves; head 2
            # alternates halves by kt parity via the swapped copy in slot 3.
            def s_operands(h, kt):
                if h < 2:
                    po = 64 * h
                    return (1, po), (0, po)
                return ((3, 0) if kt % 2 == 0 else (2, 64)), ((2, 0) if kt % 2 == 0 else (3, 64))

            def s_mm(dst, h, kt, qc):
                (lm, lp), (rm, rp) = s_operands(h, kt)
                nc.tensor.matmul(
                    dst,
                    qk_sb[lp : lp + 64, lm, KT * kt : KT * kt + KT],
                    qk_sb[rp : rp + 64, rm, QC * qc : QC * qc + QC],
                    start=True,
                    stop=True,
                    tile_position=(lp, 0),
                )

            def attn_begin(qc):
                return {
                    "qc": qc,
                    # oab: cols 0:QC = normalized [h0;h1] o, cols QC:2QC
                    # rows 0:64 = normalized h2 o
                    "oab": op_.tile([128, 2 * QC], BF16, tag="oab", name="oab"),
                    "ocO": sp.tile([128, QC], F32, tag="ocO", name="ocO", bufs=2),
                    "ocS": sp.tile([128, 2 * QC], F32, tag="ocS", name="ocS", bufs=2),
                    "pend": None,
                }

            def _flush(st, keep=0):
                # fp8 DoubleRow PV: kt pair (k0, k0+1) as the two planes.
                # PVs run `keep` supersteps behind their exp so the PE never
                # waits on an exp that was just issued.
                while len(st["pend"]) > keep:
                    _flush1(st, st["pend"].pop(0))

            def _flush1(st, pend):
                kind, k0, pt = pend
                if kind == "01":
                    pr = pt[:].rearrange("p (two x) -> p two x", two=2)
                    for h in (0, 1):
                        nc.tensor.matmul(
                            st["o_ps"][h][:],
                            v_sb[:, k0 : k0 + 2, 128 * h : 128 * h + 128],
                            pr[:, :, QC * h : QC * h + QC],
                            start=(k0 == 0),
                            stop=(k0 == NKT - 2),
                            perf_mode=DR,
                        )
                else:
                    nc.tensor.matmul(
                        st["o_ps"][2][:],
                        v_sb[:, k0 : k0 + 2, 256:384],
                        pt[:].rearrange("p (two x) -> p two x", two=2),
                        start=(k0 == 0),
                        stop=(k0 == NKT - 2),
                        perf_mode=DR,
                    )

            def attn_steps01(st, k0s, dve_b=(), filler=None):
                # heads 0/1, kt pairs; per superstep: 4 score matmuls (2
                # concurrent pairs), 2 exps, 2 DoubleRow PVs (one step late
                # so the PE never waits on the exp it just issued). dve_b
                # routes the kt+1 exp of those supersteps to the DVE
                # schraudolph; filler(j) emits PE filler (proj chunks).
                qc = st["qc"]
                o_ps = st.setdefault("o_ps", {})
                for h in (0, 1):
                    if h not in o_ps:
                        o_ps[h] = ps_acc.tile([128, QC], F32, tag="acc", name="acc")
                for j, k0 in enumerate(k0s):
                    s2a = ps.tile([128, 2 * QC], F32, tag="sa", name="sa")
                    s_mm(s2a[:, 0:QC], 0, k0, qc)
                    s_mm(s2a[:, QC : 2 * QC], 1, k0, qc)
                    s2b = ps.tile([128, 2 * QC], F32, tag="sa", name="sa")
                    s_mm(s2b[:, 0:QC], 0, k0 + 1, qc)
                    s_mm(s2b[:, QC : 2 * QC], 1, k0 + 1, qc)
                    pt = pta.tile([128, 4 * QC], F8, tag="pta", name="pta")
                    if j in dve_b:
                        # whole superstep on the DVE so the two engines never
                        # write the same pta tile concurrently
                        nc.vector.tensor_scalar(
                            pt[:, 0 : 2 * QC].bitcast(mybir.dt.int8), s2a[:],
                            SCH_K, SCH_C, Alu.mult, Alu.add,
                        )
                        nc.vector.tensor_scalar(
                            pt[:, 2 * QC : 4 * QC].bitcast(mybir.dt.int8), s2b[:],
                            SCH_K, SCH_C, Alu.mult, Alu.add,
                        )
                    else:
                        nc.scalar.activation(pt[:, 0 : 2 * QC], s2a[:], Exp, bias=actbias_sb[:])
                        nc.scalar.activation(pt[:, 2 * QC : 4 * QC], s2b[:], Exp, bias=actbias_sb[:])
                    st["pend"].append(("01", k0, pt))
                    _flush(st, keep=1)
                    if filler is not None:
                        filler(j)

            def attn_steps2(st, k0s, filler=None):
                # head 2: even kt on one partition half, odd on the other
                qc = st["qc"]
                o_ps = st.setdefault("o_ps", {})
                if 2 not in o_ps:
                    o_ps[2] = ps_acc.tile([128, QC], F32, tag="acc", name="acc")
                for idx, k0 in enumerate(k0s):
                    s2 = ps.tile([128, 2 * QC], F32, tag="sa", name="sa")
                    s_mm(s2[:, 0:QC], 2, k0, qc)
                    s_mm(s2[:, QC : 2 * QC], 2, k0 + 1, qc)
                    pt = pta.tile([128, 2 * QC], F8, tag="pt2", name="pt2")
                    if idx % 2 == 1:
                        # DVE "exp": schraudolph straight into the fp8 bit
                        # pattern — i8 = round(K*s + C), bitcast e4m3.
                        nc.vector.tensor_scalar(
                            pt[:].bitcast(mybir.dt.int8), s2[:],
                            SCH_K, SCH_C, Alu.mult, Alu.add,
                        )
                    else:
                        nc.scalar.activation(pt[:], s2[:], Exp, bias=actbias_sb[:])
                    st["pend"].append(("2", k0, pt))
                    _flush(st, keep=1)
                    if filler is not None:
                        filler(idx)

            def attn_evac01(st):
                # free heads 0/1 accumulators, assembling (o | sums) staging:
                # ocO = [h0 o ; h1 o], ocS cols 0:QC = [h0 sums ; h1 sums]
                _flush(st)
                o_ps, ocO, ocS = st["o_ps"], st["ocO"], st["ocS"]
                # sums via DVE (they feed the normalize chain next on that
                # queue); o-halves via ScalarE, which idles at this boundary
                nc.scalar.copy(ocO[0:64, :], o_ps[0][0:64, :])
                nc.scalar.copy(ocO[64:128, :], o_ps[1][0:64, :])
                nc.vector.tensor_copy(out=ocS[0:64, 0:QC], in_=o_ps[0][64:128, :])
                nc.vector.tensor_copy(out=ocS[64:128, 0:QC], in_=o_ps[1][64:128, :])
                del o_ps[0], o_ps[1]

            def attn_evac2(st):
                # h2 sums into ocS cols QC:2QC (rows 64:128 stay junk); the
                # o half stays in PSUM until the normalize multiply.
                _flush(st)
                nc.vector.tensor_copy(
                    out=st["ocS"][0:64, QC : 2 * QC], in_=st["o_ps"][2][64:128, :]
                )

            MAGIC = 0x7EF311C3

            def normalize(st, last=False):
                # rec = -(approx 1/sums): int bit-trick seed + 1 Newton step on
                # the assembled [128, 2QC] sums tile (3 heads in one chain; the
                # sign is fixed up on the host, partials are negated). The
                # seed/newton chain runs on the Pool engine to keep the DVE
                # free for exp tiles.
                ocS, ocO = st["ocS"], st["ocO"]
                seedt = sp.tile([128, 2 * QC], F32, tag="seed", name="seed", bufs=2)
                nc.vector.tensor_scalar(
                    seedt[:].bitcast(mybir.dt.int32),
                    ocS[:].bitcast(mybir.dt.int32),
                    MAGIC, -1, Alu.subtract, Alu.mult,
                )
                ut = sp.tile([128, 2 * QC], F32, tag="nru", name="nru", bufs=2)
                nc.vector.tensor_tensor(ut[:], ocS[:], seedt[:], Alu.mult)
                rect = sp.tile([128, 2 * QC], F32, tag="recip", name="recip", bufs=2)
                nc.vector.scalar_tensor_tensor(
                    rect[:], ut[:], 2.0, seedt[:], Alu.subtract, Alu.mult
                )
                oab = st["oab"]
                nc.vector.tensor_tensor(oab[:, 0:QC], ocO[:], rect[:, 0:QC], Alu.mult)
                nc.vector.tensor_tensor(
                    oab[0:64, QC : 2 * QC],
                    st["o_ps"][2][0:64, :],
                    rect[0:64, QC : 2 * QC],
                    Alu.mult,
                )
                del st["o_ps"][2]

            def proj_chunk(pst, j):
                # one (qt, column-half) chunk of the output projection for
                # pst's query slice; evictions alternate ScalarE / VectorE.
                qc, oab = pst["qc"], pst["oab"]
                qt, half = divmod(j, 2)
                if half == 0:
                    pst["ys"] = yp.tile([128, DIM], F32, tag="y", name="y")
                ys = pst["ys"]
                nsl = slice(384 * half, 384 * half + 384)
                yps = ps.tile([128, 2 * QC], F32, tag="sa", name="yps")[:, 0:384]
                nc.tensor.matmul(
                    yps[:], oab[:, 128 * qt : 128 * qt + 128], wp_sb[:, 0, nsl],
                    start=True, stop=False,
                )
                nc.tensor.matmul(
                    yps[:],
                    oab[0:64, QC + 128 * qt : QC + 128 * qt + 128],
                    wp_sb[0:64, 1, nsl],
                    start=False, stop=True,
                )
                if half == 0:
                    nc.scalar.copy(ys[:, nsl], yps[:])
                else:
                    nc.vector.tensor_copy(out=ys[:, nsl], in_=yps[:])
                    nc.gpsimd.dma_start(
                        out[QC * qc + 128 * qt : QC * qc + 128 * qt + 128, :], ys[:]
                    )

            # ---- software-pipelined schedule ----
            # Phase 1 (QK/V projections) is interleaved with attention(qc=0):
            # attention consumes K^T/V k-tiles in order, and k-tile group g
            # becomes available right after qk_phase(g)+v_tile(4g..4g+3).
            qk_phase(0)
            for kt in range(0, 4):
                v_tile(kt)
            st0 = attn_begin(0)
            attn_steps01(st0, (0, 2))
            for qq in range(1, NQC):
                qk_phase(qq)
                for kt in range(4 * qq, 4 * qq + 4):
                    v_tile(kt)
                attn_steps01(st0, (4 * qq, 4 * qq + 2))
            attn_evac01(st0)
            attn_steps2(st0, range(0, NKT, 2))
            attn_evac2(st0)
            normalize(st0)
            prev = st0
            for qc in range(1, NQC):
                st = attn_begin(qc)
                attn_steps01(st, range(0, NKT, 2), dve_b=(5, 6))
                attn_evac01(st)
                # proj(qc-1) chunks interleave with the h2 steps: the PE has
                # slack there and the evictions split across ScalarE/VectorE
                attn_steps2(
                    st, range(0, NKT, 2),
                    filler=lambda i, p=prev: proj_chunk(p, i),
                )
                attn_evac2(st)
                normalize(st, last=(qc == NQC - 1))
                prev = st
            for j in range(8):
                proj_chunk(prev, j)
    return nc


_NC_CACHE = {}


def _get_nc():
    if "nc" not in _NC_CACHE:
        _NC_CACHE["nc"] = build_kernel()
    return _NC_CACHE["nc"]


def kernel(x, qkv_w, qkv_b, proj_w, proj_b):
    x = np.asarray(x, np.float32)
    qkv_w = np.asarray(qkv_w, np.float32)
    qkv_b = np.asarray(qkv_b, np.float32)
    proj_w = np.asarray(proj_w, np.float32)
    proj_b = np.asarray(proj_b, np.float32)

    wr = qkv_w.reshape(DIM, 3, H, Dh)
    br = qkv_b.reshape(3, H, Dh)
    scale = Dh ** -0.5

    # fp8 descale vector per mt slot: mt0 all-Q, mt1 all-K, mt2 [Q2|K2]
    dsc_c = np.empty((128, 3), np.float32)
    dsc_c[:, 0] = 1.0 / SQ
    dsc_c[:, 1] = 1.0 / SK
    dsc_c[0:64, 2] = 1.0 / SQ
    dsc_c[64:128, 2] = 1.0 / SK

    in_maps = []
    for core in range(NCORES):
        b, g = divmod(core, 4)
        hs = slice(G * g, G * g + G)
        # fold softmax scale into Q; pre-scale fp8 weights out of subnormals
        wq = wr[:, 0, hs, :].reshape(DIM, G * Dh) * (scale * SQ)
        wk = wr[:, 1, hs, :].reshape(DIM, G * Dh) * SK
        wvm = wr[:, 2, hs, :].reshape(DIM, G * Dh)
        bq = br[0, hs].reshape(G * Dh) * scale
        bk = br[1, hs].reshape(G * Dh)
        # column order: mt0=[Q0|Q1], mt1=[K0|K1], mt2=[Q2|K2] (64 cols per head)
        wqk_c = np.concatenate(
            [wq[:, 0:128], wk[:, 0:128], wq[:, 128:192], wk[:, 128:192]], axis=1
        )
        bqk_c = np.concatenate([bq[0:128], bk[0:128], bq[128:192], bk[128:192]])
        xTb = np.ascontiguousarray(x[b].T)
        in_maps.append(
            {
                "xT": xTb.astype(bf16),
                "xT8": xTb.astype(e4m3).view(np.uint8),
                "wqk": np.ascontiguousarray(wqk_c).astype(e4m3).view(np.uint8),
                "bqk": np.ascontiguousarray(bqk_c),
                "dsc": dsc_c,
                "wv": np.ascontiguousarray(wvm).astype(bf16),
                "wp": np.ascontiguousarray(proj_w[64 * G * g : 64 * G * (g + 1), :]).astype(bf16),
            }
        )

    nc = _get_nc()
    res = run_bass_kernel_spmd(nc, in_maps, core_ids=list(range(NCORES)))
    _NC_CACHE["last_result"] = res

    bias_row = (br[2].reshape(DIM).astype(np.float64) @ proj_w.astype(np.float64)
                + proj_b.astype(np.float64)).astype(np.float32)
    out = np.zeros((B, N, DIM), np.float32)
    for b in range(B):
        acc = np.zeros((N, DIM), np.float64)
        for g in range(4):
            acc += res.results[4 * b + g]["out"].astype(np.float64)
        out[b] = (-acc).astype(np.float32) + bias_row
    return out

